# revision 1
# baseline (speedup 1.0000x reference)
"""Causal multi-head attention (B=4, S=2048, D=1024, H=16, HD=64) with RoPE,
distributed over 8 TRN2 NeuronCores as (batch x head-group): core c handles
batch c//2 and heads (c%2)*8..(c%2)*8+7.  Each core computes a [2048, 1024]
partial of out@wo.T restricted to its 8 heads; the host sums the two partials
per batch.

Written in raw Bass (explicit per-engine programs + manual semaphores): the
walrus build in this container rejects instructions carrying more than one
attached sync command ("Too many sync wait commands"), which rules out
TileContext; all waits here are standalone wait_ge instructions.

Per-core dataflow (all matmul operands bf16, f32 PSUM accumulation):
  - x arrives pre-transposed (xT [1024, 2048]) so Q^T/K^T come out of the PE
    with head-dim on partitions and V comes out in natural [s, feature] layout.
  - q/k weight rows are pre-permuted to evens-then-odds per head, which turns
    interleaved RoPE into: qrope = q*cos + (P2@q)*sin with P2 a constant
    128x128 block rotation applied by one PE matmul per tile.
  - scores are computed transposed (k on partitions, q on free) so the softmax
    denominator falls out of the att@V matmul: lhsT = [V | ones] gives 64
    output features plus the sum row.  No max-subtraction (scores ~ N(0,1)).
  - score tiles are processed in PAIRS into a 2-bank PSUM buffer so a single
    ScalarE exp (and a single GpSimd causal fill) covers 1024 columns.
  - normalization: DVE reciprocal of the sum row, broadcast across 64
    partitions with a free-dim-replicated SBUF->SBUF DMA; the chain is
    deferred by one head so no engine ever waits on it.
  - the output projection for a finished qc block is spread through the next
    block's head loop to avoid a serial tail.
"""

import sys

if "/opt/trn_rl_repo" not in sys.path:
    sys.path.insert(0, "/opt/trn_rl_repo")

from contextlib import ExitStack

import numpy as np
import ml_dtypes

import concourse.bass as bass
from concourse import mybir
from concourse.bass_utils import run_bass_kernel_spmd

BF16 = mybir.dt.bfloat16
F32 = mybir.dt.float32
NPBF16 = ml_dtypes.bfloat16
EXP = mybir.ActivationFunctionType.Exp

B, S, D, H, HD = 4, 2048, 1024, 16, 64
HG = 512
N_CORES = 8

_nc_cache = None
last_results = None


class _Op:
    __slots__ = ("eng", "fn", "waits", "inc", "done")

    def __init__(self, eng, fn, waits, inc):
        self.eng, self.fn, self.waits, self.inc = eng, fn, list(waits), inc
        self.done = None  # (sem_name, value) proving completion


class _Gen:
    """Pass-1 op recorder; resolves symbolic op-completion waits to semaphore
    counts, then replays each engine's program inside its Block closure."""

    ENGS = ("pe", "act", "dve", "gp", "sp")

    def __init__(self):
        self.ops = {e: [] for e in self.ENGS}

    def op(self, eng, fn, waits=(), inc=None):
        o = _Op(eng, fn, waits, inc)
        self.ops[eng].append(o)
        return o

    def resolve(self):
        for eng in self.ENGS:
            sem = "s_" + eng
            cum = 0
            cums = {}
            for o in self.ops[eng]:
                if o.inc is True:
                    cum += 1
                    o.done = (sem, cum)
                elif o.inc is not None:  # DMA: (dma_sem, 16)
                    sn, amt = o.inc
                    cums[sn] = cums.get(sn, 0) + amt
                    o.done = (sn, cums[sn])
            carry = None
            for o in reversed(self.ops[eng]):
                if o.inc is True:
                    carry = o.done
                elif o.inc is None and carry is not None:
                    o.done = carry

    def emit(self, eng_name, eng_obj, sems):
        observed = {}
        for o in self.ops[eng_name]:
            todo = {}
            for w in o.waits:
                semn, val = w.done if isinstance(w, _Op) else (w[0], w[1])
                if val > todo.get(semn, 0):
                    todo[semn] = val
            for semn, val in todo.items():
                if observed.get(semn, 0) < val:
                    eng_obj.wait_ge(sems[semn], val)
                    observed[semn] = val
            inst = o.fn(eng_obj)
            if o.inc is not None and o.inc is not True:
                inst.then_inc(sems[o.inc[0]], o.inc[1])
            elif o.inc is True:
                inst.then_inc(sems["s_" + eng_name], 1)


def _build_nc():
    nc = bass.Bass()

    xt_d = nc.declare_dram_parameter("xT", [D, S], BF16, isOutput=False)
    wq_d = nc.declare_dram_parameter("wqT", [D, HG], BF16, isOutput=False)
    wk_d = nc.declare_dram_parameter("wkT", [D, HG], BF16, isOutput=False)
    wv_d = nc.declare_dram_parameter("wvT", [D, HG], BF16, isOutput=False)
    wo_d = nc.declare_dram_parameter("woT", [HG, D], BF16, isOutput=False)
    cos_d = nc.declare_dram_parameter("cosr", [128, S], BF16, isOutput=False)
    sin_d = nc.declare_dram_parameter("sinr", [128, S], BF16, isOutput=False)
    prot_d = nc.declare_dram_parameter("protT", [128, 128], BF16, isOutput=False)
    out_d = nc.declare_dram_parameter("out", [S, D], F32, isOutput=True)

    sem_names = (["s_pe", "s_act", "s_dve", "s_gp", "s_sp"]
                 + [f"d_k{k}" for k in range(8)]
                 + ["d_wv", "d_misc", "d_rb0", "d_rb1",
                    "d_odd0", "d_odd1", "d_out0", "d_out1"])

    with ExitStack() as ctx:
        sb = lambda name, shape, dt: ctx.enter_context(nc.sbuf_tensor(name, shape, dt))

        xt = sb("xt", [128, 8, S], BF16)
        wq_sb = sb("wq_sb", [128, 8, HG], BF16)
        wk_sb = sb("wk_sb", [128, 8, HG], BF16)
        wv_sb = sb("wv_sb", [128, 8, HG], BF16)
        wo_sb = sb("wo_sb", [128, 4, D], BF16)
        cos_sb = sb("cos_sb", [128, S], BF16)
        sin_sb = sb("sin_sb", [128, S], BF16)
        prot_sb = sb("prot_sb", [128, 128], BF16)
        qropeT = sb("qropeT", [128, 4, S], BF16)
        kropeT = sb("kropeT", [128, 4, S], BF16)
        vt = sb("vt", [128, 16, 8, 65], BF16)
        attT = sb("attT", [128, 4, S], BF16)
        zeros_sb = sb("zeros_sb", [128, 1], F32)
        qt_sb = [sb(f"qt_sb{i}", [128, 512], BF16) for i in range(3)]
        t1_sb = [sb(f"t1_sb{i}", [128, 512], BF16) for i in range(2)]
        t2_sb = [sb(f"t2_sb{i}", [128, 512], BF16) for i in range(2)]
        esc_sb = [sb(f"esc_sb{i}", [128, 2, 512], BF16) for i in range(3)]
        rcp_sb = [sb(f"rcp_sb{i}", [65, 512], F32) for i in range(2)]
        rb_sb = [sb(f"rb_sb{i}", [64, 512], F32) for i in range(2)]
        odd_sb = [sb(f"odd_sb{i}", [64, 512], BF16) for i in range(2)]
        osb = [sb(f"osb{i}", [128, 512], F32) for i in range(2)]

        scp = [ctx.enter_context(nc.psum_tensor(f"scp{i}", [128, 2, 512], F32))
               for i in range(2)]
        px = [ctx.enter_context(nc.psum_tensor(f"px{i}", [128, 512], F32))
              for i in range(4)]

        sems = {n: ctx.enter_context(nc.semaphore(n)) for n in sem_names}

        g = _Gen()

        def dma(eng, dst, src, sem, waits=()):
            return g.op(eng,
                        lambda e, a=dst, b=src: e.dma_start(out=a, in_=b),
                        waits, inc=(sem, 16))

        # ---- input DMAs: (xt, wq, wk) per-kt groups gate Q/K; wv/misc later ----
        for kt in range(8):
            dma("sp", xt[:, kt, :], xt_d[kt * 128:(kt + 1) * 128, :], f"d_k{kt}")
            dma("sp", wq_sb[:, kt, :], wq_d[kt * 128:(kt + 1) * 128, :], f"d_k{kt}")
            dma("sp", wk_sb[:, kt, :], wk_d[kt * 128:(kt + 1) * 128, :], f"d_k{kt}")
        dma("sp", cos_sb[:, :], cos_d[:, :], "d_misc")
        dma("sp", sin_sb[:, :], sin_d[:, :], "d_misc")
        dma("sp", prot_sb[:, :], prot_d[:, :], "d_misc")
        for kt in range(8):
            dma("sp", wv_sb[:, kt, :], wv_d[kt * 128:(kt + 1) * 128, :], "d_wv")
        for p in range(4):
            dma("sp", wo_sb[:, p, :], wo_d[p * 128:(p + 1) * 128, :], "d_misc")
        D_KT = 48
        D_MISC_ALL = ("d_misc", 16 * 7)
        D_WV_ALL = ("d_wv", 16 * 8)

        def mm(bank_ap, lhsT, rhs, start, stop):
            return lambda e, o=bank_ap, l=lhsT, r=rhs, s=start, t=stop: e.matmul(
                o, lhsT=l, rhs=r, start=s, stop=t, skip_group_check=True)

        zeros_op = g.op("dve", lambda e: e.memset(zeros_sb[:, :], 0.0), (), inc=True)

        # 8 logical accumulator banks for phase B: the four pair-halves + px
        banks8 = ([(scp[i][:, hi, :], f"s{i}{hi}") for i in range(2) for hi in range(2)]
                  + [(px[i][:, :], f"px{i}") for i in range(4)])
        bank_war = {key: [] for _, key in banks8}
        qt_war = [[] for _ in range(3)]
        t1_war = [None, None]
        t2_war = [None, None]
        rope_ready = {}
        qtbuf = 0

        # ---- phase B1: Q^T and K^T projection + RoPE (interleaved per kt
        #      so the first qc pass chases the input-DMA stream) ----
        for qc in range(4):
            sl = slice(qc * 512, (qc + 1) * 512)
            finals = {}
            for kt in range(8):
                for wi, w_sb in enumerate((wq_sb, wk_sb)):
                    for tt in range(4):
                        bap, key = banks8[4 * wi + tt]
                        waits = [(f"d_k{kt}", D_KT)]
                        if kt == 0:
                            waits += bank_war[key]
                            bank_war[key] = []
                        op = g.op("pe", mm(bap,
                                           w_sb[:, kt, tt * 128:(tt + 1) * 128],
                                           xt[:, kt, sl], kt == 0, kt == 7),
                                  waits, inc=True if kt == 7 else None)
                        if kt == 7:
                            finals[(wi, tt)] = op
            for wi, dstT in enumerate((qropeT, kropeT)):
                for tt in range(4):
                    bap, key = banks8[4 * wi + tt]
                    bq = qtbuf % 3
                    qtbuf += 1
                    cop = g.op("act",
                               lambda e, a=qt_sb[bq], b=bap:
                               e.copy(a[:, :], b),
                               [finals[(wi, tt)]] + qt_war[bq], inc=True)
                    qt_war[bq] = []
                    # rot reuses the same bank its inputs came from (freed by cop)
                    rop = g.op("pe", mm(bap, prot_sb[:, :],
                                        qt_sb[bq][:, :], True, True),
                               [cop, D_MISC_ALL], inc=True)
                    t1waits = [cop, D_MISC_ALL]
                    if t1_war[tt % 2] is not None:
                        t1waits.append(t1_war[tt % 2])
                    t1op = g.op("dve",
                                lambda e, o=t1_sb[tt % 2], a=qt_sb[bq], c=cos_sb[:, sl]:
                                e.tensor_mul(o[:, :], a[:, :], c),
                                t1waits, inc=True)
                    t2waits = [rop]
                    if t2_war[tt % 2] is not None:
                        t2waits.append(t2_war[tt % 2])
                    t2op = g.op("dve",
                                lambda e, o=t2_sb[tt % 2], r=bap, s2=sin_sb[:, sl]:
                                e.tensor_mul(o[:, :], r, s2),
                                t2waits, inc=True)
                    bank_war[key] = [t2op]
                    addop = g.op("dve",
                                 lambda e, o=dstT[:, tt, sl], a=t1_sb[tt % 2], b=t2_sb[tt % 2]:
                                 e.tensor_add(o, a[:, :], b[:, :]),
                                 [t1op, t2op], inc=True)
                    qt_war[bq] = [rop, t1op]
                    t1_war[tt % 2] = addop
                    t2_war[tt % 2] = addop
                    rope_ready[(("q", "k")[wi], tt, qc)] = addop

        # ---- phase B2: V projection into [V | ones] layout (px banks only,
        #      leaving the score pair-banks free for early attention) ----
        vt_ready = {}
        for st in range(16):
            bap, key = banks8[4 + st % 4]
            last = None
            for kt in range(8):
                waits = [(f"d_k{kt}", D_KT), D_WV_ALL]
                if kt == 0:
                    waits += bank_war[key]
                    bank_war[key] = []
                last = g.op("pe", mm(bap,
                                     xt[:, kt, st * 128:(st + 1) * 128],
                                     wv_sb[:, kt, :], kt == 0, kt == 7),
                            waits, inc=True if kt == 7 else None)
            cop = g.op("act",
                       lambda e, o=vt[:, st, :, 0:64], i=bap:
                       e.copy(o, i.rearrange("p (h f) -> p h f", h=8)),
                       [last], inc=True)
            bank_war[key].append(cop)
            mset = g.op("dve",
                        lambda e, o=vt[:, st, :, 64:65]: e.memset(o, 1.0),
                        (), inc=True)
            vt_ready[st] = (cop, mset)

        # ---- phase C: paired scores^T -> one exp/fill per pair -> [V|1]@esc
        #      -> deferred normalization; previous block's output projection
        #      spread through the head loop ----
        esc_war = [[] for _ in range(3)]
        av_war = [bank_war["px0"], bank_war["px1"]]
        bank_war["px0"] = bank_war["px1"] = []
        prev_mul = None
        last_mul = None
        pending_norm = []
        spi = 0
        epi = 0
        avj = 0
        oddj = 0
        outi = 0
        pending_d = []

        def emit_d_group():
            nonlocal outi
            if not pending_d:
                return
            st, dc, extra = pending_d.pop(0)
            i = outi
            outi += 1
            key = f"px{2 + i % 2}"
            bap = px[2 + i % 2][:, :]
            last = None
            for pp in range(4):
                waits = []
                if pp == 0:
                    waits += bank_war[key] + extra
                    bank_war[key] = []
                last = g.op("pe", mm(bap,
                                     attT[:, pp, st * 128:(st + 1) * 128],
                                     wo_sb[:, pp, dc * 512:(dc + 1) * 512],
                                     pp == 0, pp == 3),
                            waits, inc=True if pp == 3 else None)
            outsem = f"d_out{i % 2}"
            cwaits = [last]
            if i >= 2:
                cwaits.append((outsem, 16 * (i // 2)))
            cop = g.op("dve",
                       lambda e, o=osb[i % 2], b=bap:
                       e.tensor_copy(o[:, :], b),
                       cwaits, inc=True)
            bank_war[key].append(cop)
            dma("sp", out_d[st * 128:(st + 1) * 128, dc * 512:(dc + 1) * 512],
                osb[i % 2][:, :], outsem, [cop, (outsem, 16 * (i // 2))])

        for qc in range(4):
            qsl = slice(qc * 512, (qc + 1) * 512)
            for h in range(8):
                if pending_norm:
                    pending_norm.pop(0)()
                p, half = h // 2, h % 2
                base = 64 * half
                n_kt = 4 * qc + 4
                n_pairs = n_kt // 2
                avbank = px[avj % 2]
                ready = {}
                escbuf = {}

                def emit_score_pair(pa):
                    nonlocal spi, epi
                    sp_i = spi % 2
                    spi += 1
                    eb = epi % 3
                    epi += 1
                    kt0 = 2 * pa
                    s1 = g.op("pe", mm(scp[sp_i][:, 0, :],
                                       kropeT[base:base + 64, p, kt0 * 128:(kt0 + 1) * 128],
                                       qropeT[base:base + 64, p, qsl],
                                       True, True),
                              [rope_ready[("k", p, kt0 // 4)],
                               rope_ready[("q", p, qc)]] + bank_war[f"s{sp_i}0"],
                              inc=True)
                    bank_war[f"s{sp_i}0"] = []
                    s2 = g.op("pe", mm(scp[sp_i][:, 1, :],
                                       kropeT[base:base + 64, p, (kt0 + 1) * 128:(kt0 + 2) * 128],
                                       qropeT[base:base + 64, p, qsl],
                                       True, True),
                              [rope_ready[("k", p, (kt0 + 1) // 4)]] + bank_war[f"s{sp_i}1"],
                              inc=True)
                    bank_war[f"s{sp_i}1"] = []
                    e1 = g.op("act",
                              lambda e, o=esc_sb[eb], i=scp[sp_i]:
                              e.activation(o[:, 0, :], i[:, 0, :], EXP,
                                           bias=zeros_sb[:, 0:1], scale=0.125),
                              [s1, zeros_op] + esc_war[eb], inc=True)
                    esc_war[eb] = []
                    eop = g.op("act",
                               lambda e, o=esc_sb[eb], i=scp[sp_i]:
                               e.activation(o[:, 1, :], i[:, 1, :], EXP,
                                            bias=zeros_sb[:, 0:1], scale=0.125),
                               [s2], inc=True)
                    bank_war[f"s{sp_i}0"].append(e1)
                    bank_war[f"s{sp_i}1"].append(eop)
                    fin = eop
                    if kt0 >= 4 * qc:  # diagonal pair: one fill for both halves
                        fin = g.op("gp",
                                   lambda e, o=esc_sb[eb], b=qc * 512 - kt0 * 128:
                                   e.affine_select(out=o[:, :, :], in_=o[:, :, :],
                                                   pattern=[[-128, 2], [1, 512]],
                                                   compare_op=mybir.AluOpType.is_ge,
                                                   fill=0.0, base=b,
                                                   channel_multiplier=-1),
                                   [eop], inc=True)
                    ready[pa] = fin
                    escbuf[pa] = eb

                def emit_av_pair(pa):
                    nonlocal last_av
                    eb = escbuf[pa]
                    for hi in range(2):
                        kt = 2 * pa + hi
                        waits = ([ready[pa]] if hi == 0 else []) \
                            + [vt_ready[kt][0], vt_ready[kt][1]]
                        if kt == 0:
                            waits += av_war[avj % 2]
                            av_war[avj % 2] = []
                        op = g.op("pe", mm(avbank[0:65, :], vt[:, kt, h, :],
                                           esc_sb[eb][:, hi, :],
                                           kt == 0, kt == n_kt - 1),
                                  waits, inc=True if kt == n_kt - 1 else None)
                        last_av = op
                    esc_war[eb] = [last_av]

                last_av = None
                for pa in range(min(2, n_pairs)):
                    emit_score_pair(pa)
                nxtp = 2
                for pa in range(n_pairs):
                    emit_av_pair(pa)
                    if nxtp < n_pairs:
                        emit_score_pair(nxtp)
                        nxtp += 1

                # normalization: reciprocal now; broadcast DMA + multiply are
                # deferred to the next head so nothing waits on this chain.
                myavj = avj
                rbsem = f"d_rb{myavj % 2}"
                rwaits = [last_av]
                if myavj >= 2:
                    rwaits.append((rbsem, 16 * (myavj // 2)))
                rop = g.op("dve",
                           lambda e, o=rcp_sb[myavj % 2], i=avbank:
                           e.reciprocal(o[64:65, :], i[64:65, :]),
                           rwaits, inc=True)

                def norm_chain(rop=rop, myavj=myavj, rbsem=rbsem, avbank=avbank,
                               p=p, half=half, qsl=qsl):
                    nonlocal prev_mul, last_mul, oddj
                    rsrc = rcp_sb[myavj % 2][64:65, :]
                    bcast = bass.AP(tensor=rsrc.tensor, offset=rsrc.offset,
                                    ap=[rsrc.ap[0], [0, 64], rsrc.ap[1]])
                    dma("sp", rb_sb[myavj % 2][:, :], bcast, rbsem,
                        [rop, (rbsem, 16 * (myavj // 2))])
                    mwaits = [(rbsem, 16 * (myavj // 2 + 1))]
                    if prev_mul is not None:
                        mwaits.append(prev_mul)
                    if half == 0:
                        dst = attT[0:64, p, qsl]
                    else:
                        oddsem = f"d_odd{oddj % 2}"
                        if oddj >= 2:
                            mwaits.append((oddsem, 16 * (oddj // 2)))
                        dst = odd_sb[oddj % 2][:, :]
                    mop = g.op("dve",
                               lambda e, o=dst, a=avbank, r=rb_sb[myavj % 2]:
                               e.tensor_mul(o, a[0:64, :], r[:, :]),
                               mwaits, inc=True)
                    prev_mul = mop
                    if half == 1:
                        dma("gp", attT[64:128, p, qsl], odd_sb[oddj % 2][:, :],
                            oddsem, [mop, (oddsem, 16 * (oddj // 2))])
                        oddj += 1
                    av_war[myavj % 2] = [mop]
                    last_mul = mop

                pending_norm.append(norm_chain)
                avj += 1

                emit_d_group()   # one deferred output group per head

            while pending_norm:   # flush the last head's chain at qc end
                pending_norm.pop(0)()

            extra = [last_mul, ("d_odd0", 32 * (qc + 1)),
                     ("d_odd1", 32 * (qc + 1)), D_MISC_ALL]
            for st in range(4 * qc, 4 * qc + 4):
                for dc in range(2):
                    pending_d.append((st, dc, extra))

        while pending_d:
            emit_d_group()

        g.resolve()

        with nc.allow_low_precision(reason="bf16 attention intermediates"), \
                nc.Block() as block:
            @block.tensor
            def _(eng):
                g.emit("pe", eng, sems)

            @block.scalar
            def _(eng):
                g.emit("act", eng, sems)

            @block.vector
            def _(eng):
                g.emit("dve", eng, sems)

            @block.gpsimd
            def _(eng):
                g.emit("gp", eng, sems)

            @block.sync
            def _(eng):
                g.emit("sp", eng, sems)

    return nc


def _get_nc():
    global _nc_cache
    if _nc_cache is None:
        _nc_cache = _build_nc()
    return _nc_cache


def _host_consts():
    perm = np.concatenate([
        h * HD + np.concatenate([np.arange(0, HD, 2), np.arange(1, HD, 2)])
        for h in range(8)
    ])
    P = np.zeros((64, 64), np.float32)
    P[np.arange(32), np.arange(32, 64)] = -1.0
    P[np.arange(32, 64), np.arange(32)] = 1.0
    P2 = np.zeros((128, 128), np.float32)
    P2[:64, :64] = P
    P2[64:, 64:] = P
    return perm, P2.T.astype(NPBF16)


def kernel(x, freqs_cos, freqs_sin, wq, wk, wv, wo):
    global last_results
    x = np.asarray(x, np.float32)
    cos = np.asarray(freqs_cos, np.float32)
    sin = np.asarray(freqs_sin, np.float32)
    wq = np.asarray(wq, np.float32)
    wk = np.asarray(wk, np.float32)
    wv = np.asarray(wv, np.float32)
    wo = np.asarray(wo, np.float32)

    perm, protT = _host_consts()
    cosr = np.ascontiguousarray(np.tile(cos.T, (4, 1))).astype(NPBF16)
    sinr = np.ascontiguousarray(np.tile(sin.T, (4, 1))).astype(NPBF16)

    in_maps = []
    for c in range(N_CORES):
        b, gg = c // 2, c % 2
        gsl = slice(gg * HG, (gg + 1) * HG)
        in_maps.append({
            "xT": np.ascontiguousarray(x[b].T).astype(NPBF16),
            "wqT": np.ascontiguousarray(wq[gsl][perm].T).astype(NPBF16),
            "wkT": np.ascontiguousarray(wk[gsl][perm].T).astype(NPBF16),
            "wvT": np.ascontiguousarray(wv[gsl].T).astype(NPBF16),
            "woT": np.ascontiguousarray(wo.T[gsl]).astype(NPBF16),
            "cosr": cosr,
            "sinr": sinr,
            "protT": protT,
        })

    nc = _get_nc()
    last_results = run_bass_kernel_spmd(nc, in_maps, list(range(N_CORES)))
    res = last_results.results

    out = np.empty((B, S, D), np.float32)
    for b in range(B):
        out[b] = res[2 * b]["out"] + res[2 * b + 1]["out"]
    return out



# revision 44
# speedup vs baseline: 1.5995x; 1.5995x over previous
"""Causal multi-head attention (B=4, S=2048, D=1024, H=16, HD=64) with RoPE,
distributed over 8 TRN2 NeuronCores as (batch x head-group): core c handles
batch c//2 and heads (c%2)*8..(c%2)*8+7.  Each core computes a [2048, 1024]
partial of out@wo.T restricted to its 8 heads; the host sums the two partials
per batch.

Written in raw Bass (explicit per-engine programs + manual semaphores): the
walrus build in this container rejects instructions carrying more than one
attached sync command, which rules out TileContext; all waits here are
standalone wait_ge instructions.

v2 changes over the baseline:
  - Q/K/V projections run as fp8e4m3 DoubleRow matmuls (0.5 cycles/row, 2
    contraction k-tiles per instruction).  The host splits x and the 32x
    scaled weights into (hi, lo) fp8 pairs; each projection is the 3-term
    sum x_hi*w_hi + x_lo*w_hi + x_hi*w_lo accumulated in one PSUM group,
    which matches bf16 accuracy.  The 32x weight scale is compensated by
    the exp scale (scores carry 32*32) and by setting the [V|ones] ones
    row to 32 so the softmax denominator cancels V's scale.
  - Causal trimming: the second diagonal key-pair of each query chunk only
    computes/exps/masks query columns [256:512); the first diagonal pair
    masks only columns [0:256).
  - One exp instruction per score pair ([128,2,512] in a single AP), 6 esc
    buffers, score lookahead 4.
  - B1 rope-projection units for qc+1, B2 V-projection tiles, and deferred
    output-projection groups thread through phase C's head loop on shared
    px2/px3 slot banks, keeping PE busy while the scalar engine exps.
  - PSUM->SBUF copies moved to DVE; input DMAs split across the SP and Pool
    queues; reciprocal/broadcast buffers in bf16.
"""

import sys

if "/opt/trn_rl_repo" not in sys.path:
    sys.path.insert(0, "/opt/trn_rl_repo")

from contextlib import ExitStack

import numpy as np
import ml_dtypes

import concourse.bass as bass
from concourse import mybir
from concourse.bass_utils import run_bass_kernel_spmd

BF16 = mybir.dt.bfloat16
FP8 = mybir.dt.float8e4
F32 = mybir.dt.float32
NPBF16 = ml_dtypes.bfloat16
NPFP8 = ml_dtypes.float8_e4m3
EXP = mybir.ActivationFunctionType.Exp
DR = mybir.MatmulPerfMode.DoubleRow

B, S, D, H, HD = 4, 2048, 1024, 16, 64
HG = 512
N_CORES = 8
WS = 32.0                       # fp8 weight pre-scale
EXPSCALE = 0.125 / (WS * WS)    # 1/8192, exact in f32
N_ESC = 6
LOOKAHEAD = 4

_nc_cache = None
last_results = None


class _Op:
    __slots__ = ("eng", "fn", "waits", "inc", "done")

    def __init__(self, eng, fn, waits, inc):
        self.eng, self.fn, self.waits, self.inc = eng, fn, list(waits), inc
        self.done = None  # (sem_name, value) proving completion


class _Gen:
    """Pass-1 op recorder; resolves symbolic op-completion waits to semaphore
    counts, then replays each engine's program inside its Block closure."""

    ENGS = ("pe", "act", "dve", "gp", "sp")

    def __init__(self):
        self.ops = {e: [] for e in self.ENGS}

    def op(self, eng, fn, waits=(), inc=None):
        o = _Op(eng, fn, waits, inc)
        self.ops[eng].append(o)
        return o

    def resolve(self):
        for eng in self.ENGS:
            sem = "s_" + eng
            cum = 0
            cums = {}
            for o in self.ops[eng]:
                if o.inc is True:
                    cum += 1
                    o.done = (sem, cum)
                elif o.inc is not None:  # DMA: (dma_sem, 16)
                    sn, amt = o.inc
                    cums[sn] = cums.get(sn, 0) + amt
                    o.done = (sn, cums[sn])
            carry = None
            for o in reversed(self.ops[eng]):
                if o.inc is True:
                    carry = o.done
                elif o.inc is None and carry is not None:
                    o.done = carry

    def emit(self, eng_name, eng_obj, sems):
        observed = {}
        for o in self.ops[eng_name]:
            todo = {}
            for w in o.waits:
                semn, val = w.done if isinstance(w, _Op) else (w[0], w[1])
                if val > todo.get(semn, 0):
                    todo[semn] = val
            for semn, val in todo.items():
                if observed.get(semn, 0) < val:
                    eng_obj.wait_ge(sems[semn], val)
                    observed[semn] = val
            inst = o.fn(eng_obj)
            if o.inc is not None and o.inc is not True:
                inst.then_inc(sems[o.inc[0]], o.inc[1])
            elif o.inc is True:
                inst.then_inc(sems["s_" + eng_name], 1)


def _build_nc():
    nc = bass.Bass()

    xh_d = nc.declare_dram_parameter("xh", [D, S], FP8, isOutput=False)
    xl_d = nc.declare_dram_parameter("xl", [D, S], FP8, isOutput=False)
    wqh_d = nc.declare_dram_parameter("wqh", [D, HG], FP8, isOutput=False)
    wql_d = nc.declare_dram_parameter("wql", [D, HG], FP8, isOutput=False)
    wkh_d = nc.declare_dram_parameter("wkh", [D, HG], FP8, isOutput=False)
    wkl_d = nc.declare_dram_parameter("wkl", [D, HG], FP8, isOutput=False)
    wvh_d = nc.declare_dram_parameter("wvh", [D, HG], FP8, isOutput=False)
    wvl_d = nc.declare_dram_parameter("wvl", [D, HG], FP8, isOutput=False)
    wo_d = nc.declare_dram_parameter("woT", [HG, D], BF16, isOutput=False)
    cos_d = nc.declare_dram_parameter("cosr", [128, S], BF16, isOutput=False)
    sin_d = nc.declare_dram_parameter("sinr", [128, S], BF16, isOutput=False)
    prot_d = nc.declare_dram_parameter("protT", [128, 128], BF16, isOutput=False)
    out_d = nc.declare_dram_parameter("out", [S, D], F32, isOutput=True)

    sem_names = (["s_pe", "s_act", "s_dve", "s_gp", "s_sp"]
                 + [f"d_kh{k}" for k in range(8)]
                 + [f"d_xhb{k}" for k in range(8)]
                 + [f"d_xl{k}" for k in range(8)]
                 + [f"d_xlb{k}" for k in range(8)]
                 + [f"d_wl{k}" for k in range(8)]
                 + ["d_wvh", "d_wvl", "d_pcs", "d_wo", "d_rb0", "d_rb1",
                    "d_odd0", "d_odd1", "d_out0", "d_out1"])

    with ExitStack() as ctx:
        sb = lambda name, shape, dt: ctx.enter_context(nc.sbuf_tensor(name, shape, dt))

        xh = sb("xh_sb", [128, 8, S], FP8)
        xl = sb("xl_sb", [128, 8, S], FP8)
        wqh = sb("wqh_sb", [128, 8, HG], FP8)
        wql = sb("wql_sb", [128, 8, HG], FP8)
        wkh = sb("wkh_sb", [128, 8, HG], FP8)
        wkl = sb("wkl_sb", [128, 8, HG], FP8)
        wvh = sb("wvh_sb", [128, 8, HG], FP8)
        wvl = sb("wvl_sb", [128, 8, HG], FP8)
        wo_sb = sb("wo_sb", [128, 4, D], BF16)
        cos_sb = sb("cos_sb", [128, S], BF16)
        sin_sb = sb("sin_sb", [128, S], BF16)
        prot_sb = sb("prot_sb", [128, 128], BF16)
        qropeT = sb("qropeT", [128, 4, S], BF16)
        kropeT = sb("kropeT", [128, 4, S], BF16)
        vt = sb("vt", [128, 16, 8, 65], BF16)
        attT = sb("attT", [128, 4, S], BF16)
        zeros_sb = sb("zeros_sb", [128, 1], F32)
        qt_sb = [sb(f"qt_sb{i}", [128, 512], BF16) for i in range(3)]
        t1_sb = [sb(f"t1_sb{i}", [128, 512], BF16) for i in range(2)]
        t2_sb = [sb(f"t2_sb{i}", [128, 512], BF16) for i in range(2)]
        esc_sb = [sb(f"esc_sb{i}", [128, 2, 512], BF16) for i in range(N_ESC)]
        rcp_sb = [sb(f"rcp_sb{i}", [65, 512], BF16) for i in range(2)]
        rb_sb = [sb(f"rb_sb{i}", [64, 512], BF16) for i in range(2)]
        odd_sb = [sb(f"odd_sb{i}", [64, 512], BF16) for i in range(2)]
        osb = [sb(f"osb{i}", [128, 512], F32) for i in range(2)]

        scp = [ctx.enter_context(nc.psum_tensor(f"scp{i}", [128, 2, 512], F32))
               for i in range(2)]
        px = [ctx.enter_context(nc.psum_tensor(f"px{i}", [128, 512], F32))
              for i in range(4)]

        sems = {n: ctx.enter_context(nc.semaphore(n)) for n in sem_names}

        g = _Gen()

        def dma(eng, dst, src, sem, waits=()):
            return g.op(eng,
                        lambda e, a=dst, b=src: e.dma_start(out=a, in_=b),
                        waits, inc=(sem, 16))

        # ---- input DMAs, spread over all four DMA-capable queues so the
        #      first B1 chain's kt pairs arrive as fast as possible; rope
        #      tables front-loaded so the first rot/rope ops don't wait ----
        ksl_ = lambda kt: slice(kt * 128, (kt + 1) * 128)
        SA, SB = slice(0, S // 2), slice(S // 2, S)
        dma("sp", prot_sb[:, :], prot_d[:, :], "d_pcs")
        # kt-pair-major so the first B1 chain can chase the streams; x tiles
        # split into sequence halves (the early chains only touch cols<1024):
        #   sp:  xh even + wqh         act: xh odd + wkh
        #   gp:  xl even + wql/wkl even  (act gets the odd xl/w-lo later)
        for pr in range(4):
            k0, k1 = 2 * pr, 2 * pr + 1
            dma("sp", xh[:, k0, SA], xh_d[ksl_(k0), SA], f"d_kh{k0}")
            dma("sp", wqh[:, k0, :], wqh_d[ksl_(k0), :], f"d_kh{k0}")
            dma("sp", wqh[:, k1, :], wqh_d[ksl_(k1), :], f"d_kh{k1}")
            dma("act", xh[:, k1, SA], xh_d[ksl_(k1), SA], f"d_kh{k1}")
            dma("act", wkh[:, k0, :], wkh_d[ksl_(k0), :], f"d_kh{k0}")
            dma("act", wkh[:, k1, :], wkh_d[ksl_(k1), :], f"d_kh{k1}")
            dma("gp", xl[:, k0, SA], xl_d[ksl_(k0), SA], f"d_xl{k0}")
        for kt in range(1, 8, 2):
            dma("act", xl[:, kt, SA], xl_d[ksl_(kt), SA], f"d_xl{kt}")
        for kt in range(8):
            klq = "gp" if kt % 2 == 0 else "act"
            dma(klq, wql[:, kt, :], wql_d[ksl_(kt), :], f"d_wl{kt}")
            dma(klq, wkl[:, kt, :], wkl_d[ksl_(kt), :], f"d_wl{kt}")
        dma("sp", cos_sb[:, :], cos_d[:, :], "d_pcs")
        dma("sp", sin_sb[:, :], sin_d[:, :], "d_pcs")
        for kt in range(8):
            xq = "sp" if kt % 2 == 0 else "act"
            dma(xq, xh[:, kt, SB], xh_d[ksl_(kt), SB], f"d_xhb{kt}")
            dma(xq if kt % 2 else "gp", xl[:, kt, SB], xl_d[ksl_(kt), SB],
                f"d_xlb{kt}")
        for kt in range(8):
            dma("act", wvh[:, kt, :], wvh_d[ksl_(kt), :], "d_wvh")
            dma("gp", wvl[:, kt, :], wvl_d[ksl_(kt), :], "d_wvl")
        for p in range(4):
            dma("gp", wo_sb[:, p, :], wo_d[p * 128:(p + 1) * 128, :], "d_wo")
        D_PCS = ("d_pcs", 16 * 3)
        D_WO = ("d_wo", 16 * 4)
        D_WVH = ("d_wvh", 16 * 8)
        D_WVL = ("d_wvl", 16 * 8)

        def khw(pr):
            return [(f"d_kh{2 * pr}", 48), (f"d_kh{2 * pr + 1}", 48)]

        def xlw(pr):
            return [(f"d_xl{2 * pr}", 16), (f"d_xl{2 * pr + 1}", 16)]

        def wlw(pr):
            return [(f"d_wl{2 * pr}", 32), (f"d_wl{2 * pr + 1}", 32)]

        def mm(bank_ap, lhsT, rhs, start, stop, pm=None):
            return lambda e, o=bank_ap, l=lhsT, r=rhs, s=start, t=stop, m=pm: \
                e.matmul(o, lhsT=l, rhs=r, start=s, stop=t,
                         perf_mode=m, skip_group_check=True)

        zeros_op = g.op("dve", lambda e: e.memset(zeros_sb[:, :], 0.0), (), inc=True)

        # 8 psum banks: scp halves s00,s01,s10,s11 + px0..3
        bank_of = {"s00": scp[0][:, 0, :], "s01": scp[0][:, 1, :],
                   "s10": scp[1][:, 0, :], "s11": scp[1][:, 1, :],
                   "px0": px[0][:, :], "px1": px[1][:, :],
                   "px2": px[2][:, :], "px3": px[3][:, :]}
        bank_war = {k: [] for k in bank_of}
        qt_war = [[] for _ in range(3)]
        t1_war = [None, None]
        t2_war = [None, None]
        rope_ready = {}
        qtbuf = 0
        vt_ready = {}

        # projection DR term table: (x hi/lo, w hi/lo, extra dma waits fn)
        def b1_terms(wi):
            wh = (wqh, wkh)[wi]
            wl = (wql, wkl)[wi]
            return [(xh, wh, lambda pr: khw(pr)),
                    (xl, wh, lambda pr: khw(pr) + xlw(pr)),
                    (xh, wl, lambda pr: khw(pr) + wlw(pr))]

        B2_TERMS = [(xh, wvh, lambda pr: khw(pr) + [D_WVH]),
                    (xl, wvh, lambda pr: xlw(pr) + [D_WVH]),
                    (xh, wvl, lambda pr: khw(pr) + [D_WVL])]

        def emit_b1_chain(qc, wi, tt, key):
            """12-instr DR chain x2 (col halves) -> psum bank `key` for
            (q|k, tt) of query chunk qc.  Returns state for emit_b1_rope."""
            bap = bank_of[key]
            terms = b1_terms(wi)
            last = None
            for cb in range(2):
                csl = slice(qc * 512 + cb * 256, qc * 512 + (cb + 1) * 256)
                first = True
                for ti, (xa, wa, wf) in enumerate(terms):
                    for pr in range(4):
                        waits = wf(pr)
                        if cb == 0 and first:
                            waits = waits + bank_war[key]
                            bank_war[key] = []
                        last = g.op("pe", mm(bap[:, cb * 256:(cb + 1) * 256],
                                             wa[:, 2 * pr:2 * pr + 2,
                                                tt * 128:(tt + 1) * 128],
                                             xa[:, 2 * pr:2 * pr + 2, csl],
                                             first, ti == 2 and pr == 3, DR),
                                    waits,
                                    inc=True if (cb == 1 and ti == 2 and pr == 3)
                                    else None)
                        first = False
            return (qc, wi, tt, key, last)

        def emit_b1_cop(u):
            """psum->sbuf copy of a finished B1 chain (DVE; emitted right
            after the chain so DVE starts it while PE moves on)."""
            nonlocal qtbuf
            qc, wi, tt, key, last = u
            bap = bank_of[key]
            bq = qtbuf % 3
            qtbuf += 1
            cop = g.op("dve",
                       lambda e, a=qt_sb[bq], b=bap:
                       e.tensor_copy(a[:, :], b),
                       [last] + qt_war[bq], inc=True)
            qt_war[bq] = []
            return (qc, wi, tt, key, cop, bq)

        def emit_b1_rope(u2):
            """P2 rotation + cos/sin rope combine, one slot after the copy."""
            qc, wi, tt, key, cop, bq = u2
            bap = bank_of[key]
            dstT = (qropeT, kropeT)[wi]
            sl = slice(qc * 512, (qc + 1) * 512)
            rop = g.op("pe", mm(bap, prot_sb[:, :], qt_sb[bq][:, :], True, True),
                       [cop, D_PCS], inc=True)
            t1waits = [cop, D_PCS]
            if t1_war[tt % 2] is not None:
                t1waits.append(t1_war[tt % 2])
            t1op = g.op("dve",
                        lambda e, o=t1_sb[tt % 2], a=qt_sb[bq], c=cos_sb[:, sl]:
                        e.tensor_mul(o[:, :], a[:, :], c),
                        t1waits, inc=True)
            t2waits = [rop]
            if t2_war[tt % 2] is not None:
                t2waits.append(t2_war[tt % 2])
            t2op = g.op("dve",
                        lambda e, o=t2_sb[tt % 2], r=bap, s2=sin_sb[:, sl]:
                        e.tensor_mul(o[:, :], r, s2),
                        t2waits, inc=True)
            bank_war[key] = [t2op]
            addop = g.op("dve",
                         lambda e, o=dstT[:, tt, sl], a=t1_sb[tt % 2], b=t2_sb[tt % 2]:
                         e.tensor_add(o, a[:, :], b[:, :]),
                         [t1op, t2op], inc=True)
            qt_war[bq] = [rop, t1op]
            t1_war[tt % 2] = addop
            t2_war[tt % 2] = addop
            rope_ready[(("q", "k")[wi], tt, qc)] = addop

        def emit_b2_st(st, key):
            """V projection tile st -> vt[:, st] ([V|32] layout)."""
            bap = bank_of[key]
            last = None
            for cb in range(2):
                fsl = slice(cb * 256, (cb + 1) * 256)
                first = True
                for ti, (xa, wa, wf) in enumerate(B2_TERMS):
                    for pr in range(4):
                        waits = wf(pr)
                        if cb == 0 and first:
                            waits = waits + bank_war[key]
                            bank_war[key] = []
                        last = g.op("pe", mm(bap[:, fsl],
                                             xa[:, 2 * pr:2 * pr + 2,
                                                st * 128:(st + 1) * 128],
                                             wa[:, 2 * pr:2 * pr + 2, fsl],
                                             first, ti == 2 and pr == 3, DR),
                                    waits,
                                    inc=True if (cb == 1 and ti == 2 and pr == 3)
                                    else None)
                        first = False
            cop = g.op("act",
                       lambda e, o=vt[:, st, :, 0:64], i=bap:
                       e.copy(o, i.rearrange("p (h f) -> p h f", h=8)),
                       [last], inc=True)
            bank_war[key].append(cop)
            mset = g.op("dve",
                        lambda e, o=vt[:, st, :, 64:65]: e.memset(o, WS),
                        (), inc=True)
            vt_ready[st] = (cop, mset)

        # ---- B1 for qc=0 on all 8 banks (q->scp halves, k->px);
        #      rope deferred one unit so the DVE copy overlaps PE ----
        B1_KEYS = {(0, 0): "s00", (0, 1): "s01", (0, 2): "s10", (0, 3): "s11",
                   (1, 0): "px0", (1, 1): "px1", (1, 2): "px2", (1, 3): "px3"}
        prev_u2 = None
        for tt in range(4):
            for wi in range(2):
                u2 = emit_b1_cop(emit_b1_chain(0, wi, tt, B1_KEYS[(wi, tt)]))
                if prev_u2 is not None:
                    emit_b1_rope(prev_u2)
                prev_u2 = u2
        emit_b1_rope(prev_u2)

        # ---- slot-work queues threaded through phase C head slots ----
        sb_i = 0

        def next_slot_key():
            nonlocal sb_i
            key = f"px{2 + sb_i % 2}"
            sb_i += 1
            return key

        pending_units = []   # (qc, wi, tt)
        pending_rope = []    # cop states awaiting emit_b1_rope (1-slot delay)
        pending_b2 = []      # st
        pending_d = []       # (st, dc, qc)
        pending_n1 = []      # rb-broadcast DMA issues (1-slot delay)
        pending_n2 = []      # normalization multiplies (2-slot delay)

        esc_war = [[] for _ in range(N_ESC)]
        av_war = [bank_war["px0"], bank_war["px1"]]
        bank_war["px0"] = []
        bank_war["px1"] = []
        prev_mul = None
        spi = 0
        epi = 0
        avj = 0
        oddj = 0
        outi = 0

        def emit_d_group():
            nonlocal outi
            st, dc, dqc = pending_d.pop(0)
            i = outi
            outi += 1
            key = next_slot_key()
            bap = bank_of[key]
            extra = [("d_odd0", 32 * (dqc + 1)), ("d_odd1", 32 * (dqc + 1)),
                     D_WO]
            if prev_mul is not None:
                extra.append(prev_mul)
            last = None
            for pp in range(4):
                waits = []
                if pp == 0:
                    waits = bank_war[key] + extra
                    bank_war[key] = []
                last = g.op("pe", mm(bap,
                                     attT[:, pp, st * 128:(st + 1) * 128],
                                     wo_sb[:, pp, dc * 512:(dc + 1) * 512],
                                     pp == 0, pp == 3),
                            waits, inc=True if pp == 3 else None)
            outsem = f"d_out{i % 2}"
            cwaits = [last]
            if i >= 2:
                cwaits.append((outsem, 16 * (i // 2)))
            cop = g.op("dve",
                       lambda e, o=osb[i % 2], b=bap:
                       e.tensor_copy(o[:, :], b),
                       cwaits, inc=True)
            bank_war[key].append(cop)
            dma("sp", out_d[st * 128:(st + 1) * 128, dc * 512:(dc + 1) * 512],
                osb[i % 2][:, :], outsem, [cop, (outsem, 16 * (i // 2))])

        # ---- B2 st0..7 upfront (st0..3 needed by qc0's AV; st4..7 keep the
        #      window-0 slots down to one B2 tenant each) ----
        for st in range(8):
            emit_b2_st(st, next_slot_key())
        pending_b2 = list(range(8, 16))
        pending_units = [(1, wi, tt) for tt in range(4) for wi in range(2)]

        # ---- phase C: per query chunk, 8 heads; slot work threaded in ----
        for qc in range(4):
            qsl = slice(qc * 512, (qc + 1) * 512)
            for slot_idx, h in enumerate((1, 0, 3, 2, 5, 4, 7, 6)):
                if pending_n1:
                    pending_n1.pop(0)()
                if len(pending_n2) >= (1 if qc == 3 and slot_idx >= 5 else 2):
                    pending_n2.pop(0)()

                p, half = h // 2, h % 2
                base = 64 * half
                n_kt = 4 * qc + 4
                n_pairs = 2 * qc + 2
                hp = 2 * qc + 1          # half (trimmed) diagonal pair
                avbank = px[avj % 2]
                ready = {}
                escbuf = {}

                def emit_score_pair(pa):
                    nonlocal spi, epi
                    sp_i = spi % 2
                    spi += 1
                    eb = epi % N_ESC
                    epi += 1
                    kt0 = 2 * pa
                    cs = slice(256, 512) if pa == hp else slice(0, 512)
                    qs = slice(qc * 512 + cs.start, qc * 512 + cs.stop)
                    s1 = g.op("pe", mm(scp[sp_i][:, 0, cs],
                                       kropeT[base:base + 64, p,
                                              kt0 * 128:(kt0 + 1) * 128],
                                       qropeT[base:base + 64, p, qs],
                                       True, True),
                              [rope_ready[("k", p, kt0 // 4)],
                               rope_ready[("q", p, qc)]] + bank_war[f"s{sp_i}0"],
                              inc=True)
                    bank_war[f"s{sp_i}0"] = []
                    s2 = g.op("pe", mm(scp[sp_i][:, 1, cs],
                                       kropeT[base:base + 64, p,
                                              (kt0 + 1) * 128:(kt0 + 2) * 128],
                                       qropeT[base:base + 64, p, qs],
                                       True, True),
                              [rope_ready[("k", p, (kt0 + 1) // 4)]]
                              + bank_war[f"s{sp_i}1"],
                              inc=True)
                    bank_war[f"s{sp_i}1"] = []
                    eop = g.op("act",
                               lambda e, o=esc_sb[eb], i=scp[sp_i], c=cs:
                               e.activation(o[:, :, c], i[:, :, c], EXP,
                                            bias=zeros_sb[:, 0:1],
                                            scale=EXPSCALE),
                               [s2, zeros_op] + esc_war[eb], inc=True)
                    esc_war[eb] = []
                    bank_war[f"s{sp_i}0"].append(eop)
                    bank_war[f"s{sp_i}1"].append(eop)
                    fin = eop
                    if pa >= 2 * qc:  # diagonal pair: triangular fill
                        acs = slice(0, 256) if pa == 2 * qc else slice(256, 512)
                        fin = g.op("gp",
                                   lambda e, o=esc_sb[eb], c=acs:
                                   e.affine_select(out=o[:, :, c], in_=o[:, :, c],
                                                   pattern=[[-128, 2], [1, 256]],
                                                   compare_op=mybir.AluOpType.is_ge,
                                                   fill=0.0, base=0,
                                                   channel_multiplier=-1),
                                   [eop], inc=True)
                    ready[pa] = fin
                    escbuf[pa] = eb

                def emit_av_pair(pa):
                    nonlocal last_av
                    eb = escbuf[pa]
                    cs = slice(256, 512) if pa == hp else slice(0, 512)
                    for hi in range(2):
                        kt = 2 * pa + hi
                        waits = ([ready[pa]] if hi == 0 else []) \
                            + [vt_ready[kt][0], vt_ready[kt][1]]
                        if kt == 0:
                            waits = waits + av_war[avj % 2]
                            av_war[avj % 2] = []
                        op = g.op("pe", mm(avbank[0:65, cs], vt[:, kt, h, :],
                                           esc_sb[eb][:, hi, cs],
                                           kt == 0, kt == n_kt - 1),
                                  waits, inc=True if kt == n_kt - 1 else None)
                        last_av = op
                    esc_war[eb] = [last_av]

                last_av = None
                for pa in range(min(2, n_pairs)):
                    emit_score_pair(pa)
                # threaded PE work sits between the scores and the AVs so the
                # exp+mask latency of the first pairs is hidden behind it, and
                # the rope/chain PE ops trail the DVE results they consume
                if pending_rope:
                    emit_b1_rope(pending_rope.pop(0))
                if pending_b2:
                    emit_b2_st(pending_b2.pop(0), next_slot_key())
                if pending_d and slot_idx >= 1:
                    emit_d_group()
                if n_pairs > 2:
                    emit_score_pair(2)
                # defer half of qc3's units into window 3, which otherwise
                # has no threaded PE work to cover the scalar engine's exps
                if pending_units and not (qc == 2 and slot_idx >= 4):
                    uqc, uwi, utt = pending_units.pop(0)
                    pending_rope.append(emit_b1_cop(
                        emit_b1_chain(uqc, uwi, utt, next_slot_key())))
                for pa in range(3, min(LOOKAHEAD, n_pairs)):
                    emit_score_pair(pa)
                nxtp = LOOKAHEAD
                for pa in range(n_pairs):
                    emit_av_pair(pa)
                    if nxtp < n_pairs:
                        emit_score_pair(nxtp)
                        nxtp += 1

                # normalization pipeline: reciprocal now; rb-broadcast DMA one
                # slot later; multiply (+ odd-half DMA) two slots later, so
                # the DMA latency never blocks the in-order DVE queue.
                myavj = avj
                rbsem = f"d_rb{myavj % 2}"
                rwaits = [last_av]
                if myavj >= 2:
                    rwaits.append((rbsem, 16 * (myavj // 2)))
                rop = g.op("dve",
                           lambda e, o=rcp_sb[myavj % 2], i=avbank:
                           e.reciprocal(o[64:65, :], i[64:65, :]),
                           rwaits, inc=True)

                def norm_rb(rop=rop, myavj=myavj, rbsem=rbsem):
                    rsrc = rcp_sb[myavj % 2][64:65, :]
                    bcast = bass.AP(tensor=rsrc.tensor, offset=rsrc.offset,
                                    ap=[rsrc.ap[0], [0, 64], rsrc.ap[1]])
                    dma("sp", rb_sb[myavj % 2][:, :], bcast, rbsem,
                        [rop, (rbsem, 16 * (myavj // 2))])

                def norm_mul(myavj=myavj, rbsem=rbsem, avbank=avbank,
                             p=p, half=half, qsl=qsl):
                    nonlocal prev_mul, oddj
                    mwaits = [(rbsem, 16 * (myavj // 2 + 1))]
                    if prev_mul is not None:
                        mwaits.append(prev_mul)
                    if half == 0:
                        dst = attT[0:64, p, qsl]
                    else:
                        oddsem = f"d_odd{oddj % 2}"
                        if oddj >= 2:
                            mwaits.append((oddsem, 16 * (oddj // 2)))
                        dst = odd_sb[oddj % 2][:, :]
                    mop = g.op("dve",
                               lambda e, o=dst, a=avbank, r=rb_sb[myavj % 2]:
                               e.tensor_mul(o, a[0:64, :], r[:, :]),
                               mwaits, inc=True)
                    prev_mul = mop
                    if half == 1:
                        dma("gp", attT[64:128, p, qsl], odd_sb[oddj % 2][:, :],
                            oddsem, [mop, (oddsem, 16 * (oddj // 2))])
                        oddj += 1
                    av_war[myavj % 2] = [mop]

                pending_n1.append(norm_rb)
                pending_n2.append(norm_mul)
                avj += 1

                if qc == 3 and slot_idx >= 4 and pending_n1:
                    pending_n1.pop(0)()

            if qc + 2 <= 3:
                pending_units.extend(
                    (qc + 2, wi, tt) for tt in range(4) for wi in range(2))

            for st in range(4 * qc, 4 * qc + 4):
                for dc in range(2):
                    pending_d.append((st, dc, qc))

        while pending_n1:
            pending_n1.pop(0)()
        while pending_n2:
            pending_n2.pop(0)()
        while pending_rope:
            emit_b1_rope(pending_rope.pop(0))
        while pending_d:
            emit_d_group()

        g.resolve()

        with nc.allow_low_precision(reason="fp8/bf16 attention intermediates"), \
                nc.Block() as block:
            @block.tensor
            def _(eng):
                g.emit("pe", eng, sems)

            @block.scalar
            def _(eng):
                g.emit("act", eng, sems)

            @block.vector
            def _(eng):
                g.emit("dve", eng, sems)

            @block.gpsimd
            def _(eng):
                g.emit("gp", eng, sems)

            @block.sync
            def _(eng):
                g.emit("sp", eng, sems)

    return nc


def _get_nc():
    global _nc_cache
    if _nc_cache is None:
        _nc_cache = _build_nc()
    return _nc_cache


def _host_consts():
    perm = np.concatenate([
        h * HD + np.concatenate([np.arange(0, HD, 2), np.arange(1, HD, 2)])
        for h in range(8)
    ])
    P = np.zeros((64, 64), np.float32)
    P[np.arange(32), np.arange(32, 64)] = -1.0
    P[np.arange(32, 64), np.arange(32)] = 1.0
    P2 = np.zeros((128, 128), np.float32)
    P2[:64, :64] = P
    P2[64:, 64:] = P
    return perm, P2.T.astype(NPBF16)


def _split8(a):
    """f32 -> (hi, lo) fp8e4m3 with lo = fp8(a - hi)."""
    hi = np.ascontiguousarray(a).astype(NPFP8)
    lo = (a - hi.astype(np.float32)).astype(NPFP8)
    return hi, lo


def kernel(x, freqs_cos, freqs_sin, wq, wk, wv, wo):
    global last_results
    x = np.asarray(x, np.float32)
    cos = np.asarray(freqs_cos, np.float32)
    sin = np.asarray(freqs_sin, np.float32)
    wq = np.asarray(wq, np.float32)
    wk = np.asarray(wk, np.float32)
    wv = np.asarray(wv, np.float32)
    wo = np.asarray(wo, np.float32)

    perm, protT = _host_consts()
    cosr = np.ascontiguousarray(np.tile(cos.T, (4, 1))).astype(NPBF16)
    sinr = np.ascontiguousarray(np.tile(sin.T, (4, 1))).astype(NPBF16)

    xs = [_split8(np.ascontiguousarray(x[b].T)) for b in range(B)]
    wqs, wks, wvs, wos = {}, {}, {}, {}
    for gg in range(2):
        gsl = slice(gg * HG, (gg + 1) * HG)
        wqs[gg] = _split8(np.ascontiguousarray((wq[gsl][perm] * WS).T))
        wks[gg] = _split8(np.ascontiguousarray((wk[gsl][perm] * WS).T))
        wvs[gg] = _split8(np.ascontiguousarray((wv[gsl] * WS).T))
        wos[gg] = np.ascontiguousarray(wo.T[gsl]).astype(NPBF16)

    in_maps = []
    for c in range(N_CORES):
        b, gg = c // 2, c % 2
        in_maps.append({
            "xh": xs[b][0], "xl": xs[b][1],
            "wqh": wqs[gg][0], "wql": wqs[gg][1],
            "wkh": wks[gg][0], "wkl": wks[gg][1],
            "wvh": wvs[gg][0], "wvl": wvs[gg][1],
            "woT": wos[gg],
            "cosr": cosr,
            "sinr": sinr,
            "protT": protT,
        })

    nc = _get_nc()
    last_results = run_bass_kernel_spmd(nc, in_maps, list(range(N_CORES)))
    res = last_results.results

    out = np.empty((B, S, D), np.float32)
    for b in range(B):
        out[b] = res[2 * b]["out"] + res[2 * b + 1]["out"]
    return out


# revision 83
# speedup vs baseline: 1.6566x; 1.0357x over previous
"""Causal multi-head attention (B=4, S=2048, D=1024, H=16, HD=64) with RoPE,
distributed over 8 TRN2 NeuronCores as (batch x head-group): core c handles
batch c//2 and heads (c%2)*8..(c%2)*8+7.  Each core computes a [2048, 1024]
partial of out@wo.T restricted to its 8 heads; the host sums the two partials
per batch.

Written in raw Bass (explicit per-engine programs + manual semaphores): the
walrus build in this container rejects instructions carrying more than one
attached sync command, which rules out TileContext; all waits here are
standalone wait_ge instructions.

v2 changes over the baseline (440.9us -> 266.1us on the CoreSim cost model):
  - Q/K/V projections run as fp8e4m3 DoubleRow matmuls (0.5 cycles/row, 2
    contraction k-tiles per instruction = 4x bf16 throughput).  The host
    splits x and the 32x-scaled weights into (hi, lo) fp8 pairs; each
    projection is the 3-term sum x_hi*w_hi + x_lo*w_hi + x_hi*w_lo
    accumulated in one PSUM group, which matches bf16 accuracy (the 32x
    scale keeps the small weights out of fp8's denormal range).  The scale
    is compensated by the exp scale (scores carry 32*32) and by setting
    the [V|ones] ones row to 32 so the softmax denominator cancels V's.
    Scores/AV/out-projection stay bf16: quantizing q/k/att to fp8 puts
    ~2.5% multiplicative noise straight on the output and fails the gate.
  - Causal trimming: the second diagonal key-pair of each query chunk only
    computes/exps/masks query columns [256:512); the first diagonal pair
    masks only columns [0:256).
  - One exp instruction per score pair ([128,2,512] in a single AP), 6 esc
    buffers, score lookahead 4.
  - Software-pipelined slot schedule: each head-slot of phase C threads in
    B1 rope-projection chains for qc+1, B2 V-projection tiles, and the
    previous chunk's output-projection groups on shared px2/px3 banks, so
    PE keeps running while the scalar engine exps.  The B1 psum->sbuf
    copy, rotation, and rope combine are deferred one slot behind the
    chain; the normalization runs as a 3-stage pipeline (reciprocal ->
    rb-broadcast DMA one slot later -> multiply two slots later) so DMA
    latency never blocks the in-order DVE queue.  Odd-half heads run
    first so the last head's attT write is the direct (non-DMA) half.
  - Input DMAs spread kt-pair-major across the SP/Act/Pool queues with
    the x tiles split into sequence halves, separately semaphored, so the
    first projection chains only gate on the data they read.
"""

import sys

if "/opt/trn_rl_repo" not in sys.path:
    sys.path.insert(0, "/opt/trn_rl_repo")

from contextlib import ExitStack

import numpy as np
import ml_dtypes

import concourse.bass as bass
from concourse import mybir
from concourse.bass_utils import run_bass_kernel_spmd

BF16 = mybir.dt.bfloat16
FP8 = mybir.dt.float8e4
F32 = mybir.dt.float32
NPBF16 = ml_dtypes.bfloat16
NPFP8 = ml_dtypes.float8_e4m3
EXP = mybir.ActivationFunctionType.Exp
DR = mybir.MatmulPerfMode.DoubleRow

B, S, D, H, HD = 4, 2048, 1024, 16, 64
HG = 512
N_CORES = 8
WS = 32.0                       # fp8 weight pre-scale
EXPSCALE = 0.125 / (WS * WS)    # 1/8192, exact in f32
N_ESC = 6
LOOKAHEAD = 4

_nc_cache = None
last_results = None


class _Op:
    __slots__ = ("eng", "fn", "waits", "inc", "done")

    def __init__(self, eng, fn, waits, inc):
        self.eng, self.fn, self.waits, self.inc = eng, fn, list(waits), inc
        self.done = None  # (sem_name, value) proving completion


class _Gen:
    """Pass-1 op recorder; resolves symbolic op-completion waits to semaphore
    counts, then replays each engine's program inside its Block closure."""

    ENGS = ("pe", "act", "dve", "gp", "sp")

    def __init__(self):
        self.ops = {e: [] for e in self.ENGS}

    def op(self, eng, fn, waits=(), inc=None):
        o = _Op(eng, fn, waits, inc)
        self.ops[eng].append(o)
        return o

    def resolve(self):
        for eng in self.ENGS:
            sem = "s_" + eng
            cum = 0
            cums = {}
            for o in self.ops[eng]:
                if o.inc is True:
                    cum += 1
                    o.done = (sem, cum)
                elif o.inc is not None:  # DMA: (dma_sem, 16)
                    sn, amt = o.inc
                    cums[sn] = cums.get(sn, 0) + amt
                    o.done = (sn, cums[sn])
            carry = None
            for o in reversed(self.ops[eng]):
                if o.inc is True:
                    carry = o.done
                elif o.inc is None and carry is not None:
                    o.done = carry

    def emit(self, eng_name, eng_obj, sems):
        observed = {}
        for o in self.ops[eng_name]:
            todo = {}
            for w in o.waits:
                semn, val = w.done if isinstance(w, _Op) else (w[0], w[1])
                if val > todo.get(semn, 0):
                    todo[semn] = val
            for semn, val in todo.items():
                if observed.get(semn, 0) < val:
                    eng_obj.wait_ge(sems[semn], val)
                    observed[semn] = val
            inst = o.fn(eng_obj)
            if o.inc is not None and o.inc is not True:
                inst.then_inc(sems[o.inc[0]], o.inc[1])
            elif o.inc is True:
                inst.then_inc(sems["s_" + eng_name], 1)


def _build_nc():
    nc = bass.Bass()

    xh_d = nc.declare_dram_parameter("xh", [D, S], FP8, isOutput=False)
    xl_d = nc.declare_dram_parameter("xl", [D, S], FP8, isOutput=False)
    wqh_d = nc.declare_dram_parameter("wqh", [D, HG], FP8, isOutput=False)
    wql_d = nc.declare_dram_parameter("wql", [D, HG], FP8, isOutput=False)
    wkh_d = nc.declare_dram_parameter("wkh", [D, HG], FP8, isOutput=False)
    wkl_d = nc.declare_dram_parameter("wkl", [D, HG], FP8, isOutput=False)
    wvh_d = nc.declare_dram_parameter("wvh", [D, HG], FP8, isOutput=False)
    wvl_d = nc.declare_dram_parameter("wvl", [D, HG], FP8, isOutput=False)
    wo_d = nc.declare_dram_parameter("woT", [HG, D], BF16, isOutput=False)
    cos_d = nc.declare_dram_parameter("cosr", [128, S], BF16, isOutput=False)
    sin_d = nc.declare_dram_parameter("sinr", [128, S], BF16, isOutput=False)
    prot_d = nc.declare_dram_parameter("protT", [128, 128], BF16, isOutput=False)
    out_d = nc.declare_dram_parameter("out", [S, D], F32, isOutput=True)

    sem_names = (["s_pe", "s_act", "s_dve", "s_gp", "s_sp"]
                 + [f"d_kh{k}" for k in range(8)]
                 + [f"d_khc{k}" for k in range(8)]
                 + [f"d_xhb{k}" for k in range(8)]
                 + [f"d_xlc{k}" for k in range(8)]
                 + [f"d_xl{k}" for k in range(8)]
                 + [f"d_xlb{k}" for k in range(8)]
                 + [f"d_wl{k}" for k in range(8)]
                 + ["d_wvh", "d_wvl", "d_cs", "d_pcs", "d_wo", "d_rb0", "d_rb1",
                    "d_odd0", "d_odd1", "d_out0", "d_out1"])

    with ExitStack() as ctx:
        sb = lambda name, shape, dt: ctx.enter_context(nc.sbuf_tensor(name, shape, dt))

        xh = sb("xh_sb", [128, 8, S], FP8)
        xl = sb("xl_sb", [128, 8, S], FP8)
        wqh = sb("wqh_sb", [128, 8, HG], FP8)
        wql = sb("wql_sb", [128, 8, HG], FP8)
        wkh = sb("wkh_sb", [128, 8, HG], FP8)
        wkl = sb("wkl_sb", [128, 8, HG], FP8)
        wvh = sb("wvh_sb", [128, 8, HG], FP8)
        wvl = sb("wvl_sb", [128, 8, HG], FP8)
        wo_sb = sb("wo_sb", [128, 4, D], BF16)
        cos_sb = sb("cos_sb", [128, S], BF16)
        sin_sb = sb("sin_sb", [128, S], BF16)
        prot_sb = sb("prot_sb", [128, 128], BF16)
        qropeT = sb("qropeT", [128, 4, S], BF16)
        kropeT = sb("kropeT", [128, 4, S], BF16)
        vt = sb("vt", [128, 16, 8, 65], BF16)
        attT = sb("attT", [128, 4, S], BF16)
        zeros_sb = sb("zeros_sb", [128, 1], F32)
        qt_sb = [sb(f"qt_sb{i}", [128, 512], BF16) for i in range(3)]
        t1_sb = [sb(f"t1_sb{i}", [128, 512], BF16) for i in range(2)]
        t2_sb = [sb(f"t2_sb{i}", [128, 512], BF16) for i in range(2)]
        esc_sb = [sb(f"esc_sb{i}", [128, 2, 512], BF16) for i in range(N_ESC)]
        rcp_sb = [sb(f"rcp_sb{i}", [65, 512], BF16) for i in range(2)]
        rb_sb = [sb(f"rb_sb{i}", [64, 512], BF16) for i in range(2)]
        odd_sb = [sb(f"odd_sb{i}", [64, 512], BF16) for i in range(2)]
        osb = [sb(f"osb{i}", [128, 512], F32) for i in range(2)]

        scp = [ctx.enter_context(nc.psum_tensor(f"scp{i}", [128, 2, 512], F32))
               for i in range(2)]
        px = [ctx.enter_context(nc.psum_tensor(f"px{i}", [128, 512], F32))
              for i in range(4)]

        sems = {n: ctx.enter_context(nc.semaphore(n)) for n in sem_names}

        g = _Gen()

        def dma(eng, dst, src, sem, waits=()):
            return g.op(eng,
                        lambda e, a=dst, b=src: e.dma_start(out=a, in_=b),
                        waits, inc=(sem, 16))

        # ---- input DMAs, spread over all four DMA-capable queues so the
        #      first B1 chain's kt pairs arrive as fast as possible; rope
        #      tables front-loaded so the first rot/rope ops don't wait ----
        ksl_ = lambda kt: slice(kt * 128, (kt + 1) * 128)
        SQ, SC = slice(0, S // 4), slice(S // 4, S // 2)
        SB = slice(S // 2, S)
        # kt-pair-major so the first B1 chain can chase the streams; x tiles
        # split into sequence halves (the early chains only touch cols<1024):
        #   sp:  xh even + wqh         act: xh odd + wkh
        #   gp:  xl even + wql/wkl even  (act gets the odd xl/w-lo later)
        for pr in range(4):
            k0, k1 = 2 * pr, 2 * pr + 1
            dma("sp", xh[:, k0, SQ], xh_d[ksl_(k0), SQ], f"d_kh{k0}")
            dma("sp", wqh[:, k0, :], wqh_d[ksl_(k0), :], f"d_kh{k0}")
            dma("sp", wqh[:, k1, :], wqh_d[ksl_(k1), :], f"d_kh{k1}")
            dma("act", wkh[:, k0, :], wkh_d[ksl_(k0), :], f"d_kh{k0}")
            dma("act", xh[:, k1, SQ], xh_d[ksl_(k1), SQ], f"d_kh{k1}")
            dma("act", wkh[:, k1, :], wkh_d[ksl_(k1), :], f"d_kh{k1}")
            dma("gp", xl[:, k0, SQ], xl_d[ksl_(k0), SQ], f"d_xl{k0}")
        for kt in range(1, 8, 2):
            dma("act", xl[:, kt, SQ], xl_d[ksl_(kt), SQ], f"d_xl{kt}")
        for kt in range(8):
            klq = "gp" if kt % 2 == 0 else "act"
            dma(klq, wql[:, kt, :], wql_d[ksl_(kt), :], f"d_wl{kt}")
            dma(klq, wkl[:, kt, :], wkl_d[ksl_(kt), :], f"d_wl{kt}")
        for kt in range(8):
            xq = "sp" if kt % 2 == 0 else "act"
            dma(xq, xh[:, kt, SC], xh_d[ksl_(kt), SC], f"d_khc{kt}")
            dma(xq if kt % 2 else "gp", xl[:, kt, SC], xl_d[ksl_(kt), SC],
                f"d_xlc{kt}")
        dma("sp", prot_sb[:, :], prot_d[:, :], "d_pcs")
        dma("gp", cos_sb[:, :], cos_d[:, :], "d_cs")
        dma("gp", sin_sb[:, :], sin_d[:, :], "d_cs")
        for kt in range(8):
            xq = "sp" if kt % 2 == 0 else "act"
            dma(xq, xh[:, kt, SB], xh_d[ksl_(kt), SB], f"d_xhb{kt}")
            dma(xq if kt % 2 else "gp", xl[:, kt, SB], xl_d[ksl_(kt), SB],
                f"d_xlb{kt}")
        for kt in range(8):
            dma("act", wvh[:, kt, :], wvh_d[ksl_(kt), :], "d_wvh")
            dma("gp", wvl[:, kt, :], wvl_d[ksl_(kt), :], "d_wvl")
        for p in range(4):
            dma("gp", wo_sb[:, p, :], wo_d[p * 128:(p + 1) * 128, :], "d_wo")
        D_PCS = ("d_pcs", 16)
        D_COS = ("d_cs", 32)
        D_SIN = ("d_cs", 32)
        D_WO = ("d_wo", 16 * 4)
        D_WVH = ("d_wvh", 16 * 8)
        D_WVL = ("d_wvl", 16 * 8)

        def khw(pr, c=False, b=False):
            w = [(f"d_kh{2 * pr}", 48), (f"d_kh{2 * pr + 1}", 48)]
            if c:
                w += [(f"d_khc{2 * pr}", 16), (f"d_khc{2 * pr + 1}", 16)]
            if b:
                w += [(f"d_xhb{2 * pr}", 16), (f"d_xhb{2 * pr + 1}", 16)]
            return w

        def xlw(pr, c=False, b=False):
            w = [(f"d_xl{2 * pr}", 16), (f"d_xl{2 * pr + 1}", 16)]
            if c:
                w += [(f"d_xlc{2 * pr}", 16), (f"d_xlc{2 * pr + 1}", 16)]
            if b:
                w += [(f"d_xlb{2 * pr}", 16), (f"d_xlb{2 * pr + 1}", 16)]
            return w

        def wlw(pr):
            return [(f"d_wl{2 * pr}", 32), (f"d_wl{2 * pr + 1}", 32)]

        def mm(bank_ap, lhsT, rhs, start, stop, pm=None):
            return lambda e, o=bank_ap, l=lhsT, r=rhs, s=start, t=stop, m=pm: \
                e.matmul(o, lhsT=l, rhs=r, start=s, stop=t,
                         perf_mode=m, skip_group_check=True)

        zeros_op = g.op("dve", lambda e: e.memset(zeros_sb[:, :], 0.0), (), inc=True)

        # 8 psum banks: scp halves s00,s01,s10,s11 + px0..3
        bank_of = {"s00": scp[0][:, 0, :], "s01": scp[0][:, 1, :],
                   "s10": scp[1][:, 0, :], "s11": scp[1][:, 1, :],
                   "px0": px[0][:, :], "px1": px[1][:, :],
                   "px2": px[2][:, :], "px3": px[3][:, :]}
        bank_war = {k: [] for k in bank_of}
        qt_war = [[] for _ in range(3)]
        t1_war = [None, None]
        t2_war = [None, None]
        rope_ready = {}
        qtbuf = 0
        vt_ready = {}

        # projection DR term table: (x hi/lo, w hi/lo, extra dma waits fn);
        # `b` asks for the second sequence half of the x tiles (cols >= 1024)
        def b1_terms(wi, c, b):
            wh = (wqh, wkh)[wi]
            wl = (wql, wkl)[wi]
            return [(xh, wh, lambda pr: khw(pr, c, b)),
                    (xl, wh, lambda pr: khw(pr) + xlw(pr, c, b)),
                    (xh, wl, lambda pr: khw(pr, c, b) + wlw(pr))]

        def b2_terms(c, b):
            return [(xh, wvh, lambda pr: khw(pr, c, b) + [D_WVH]),
                    (xl, wvh, lambda pr: xlw(pr, c, b) + [D_WVH]),
                    (xh, wvl, lambda pr: khw(pr, c, b) + [D_WVL])]

        def emit_b1_chain(qc, wi, tt, key):
            """12-instr DR chain x2 (col halves) -> psum bank `key` for
            (q|k, tt) of query chunk qc.  Returns state for emit_b1_rope."""
            bap = bank_of[key]
            terms = b1_terms(wi, qc == 1, qc >= 2)
            last = None
            for cb in range(2):
                csl = slice(qc * 512 + cb * 256, qc * 512 + (cb + 1) * 256)
                first = True
                for ti, (xa, wa, wf) in enumerate(terms):
                    for pr in range(4):
                        waits = wf(pr)
                        if cb == 0 and first:
                            waits = waits + bank_war[key]
                            bank_war[key] = []
                        last = g.op("pe", mm(bap[:, cb * 256:(cb + 1) * 256],
                                             wa[:, 2 * pr:2 * pr + 2,
                                                tt * 128:(tt + 1) * 128],
                                             xa[:, 2 * pr:2 * pr + 2, csl],
                                             first, ti == 2 and pr == 3, DR),
                                    waits,
                                    inc=True if (cb == 1 and ti == 2 and pr == 3)
                                    else None)
                        first = False
            return (qc, wi, tt, key, last)

        def emit_b1_cop(u):
            """psum->sbuf copy of a finished B1 chain (DVE; emitted right
            after the chain so DVE starts it while PE moves on)."""
            nonlocal qtbuf
            qc, wi, tt, key, last = u
            bap = bank_of[key]
            bq = qtbuf % 3
            qtbuf += 1
            cop = g.op("dve",
                       lambda e, a=qt_sb[bq], b=bap:
                       e.tensor_copy(a[:, :], b),
                       [last] + qt_war[bq], inc=True)
            qt_war[bq] = []
            return (qc, wi, tt, key, cop, bq)

        def emit_b1_rope(u2):
            """P2 rotation + cos/sin rope combine, one slot after the copy
            (rotation signs live in the host-negated sin table)."""
            qc, wi, tt, key, cop, bq = u2
            bap = bank_of[key]
            dstT = (qropeT, kropeT)[wi]
            sl = slice(qc * 512, (qc + 1) * 512)
            rop = g.op("pe", mm(bap, prot_sb[:, :], qt_sb[bq][:, :], True, True),
                       [cop, D_PCS], inc=True)
            t1waits = [cop, D_COS]
            if t1_war[tt % 2] is not None:
                t1waits.append(t1_war[tt % 2])
            t1op = g.op("dve",
                        lambda e, o=t1_sb[tt % 2], a=qt_sb[bq], c=cos_sb[:, sl]:
                        e.tensor_mul(o[:, :], a[:, :], c),
                        t1waits, inc=True)
            t2waits = [rop, D_SIN]
            if t2_war[tt % 2] is not None:
                t2waits.append(t2_war[tt % 2])
            t2op = g.op("dve",
                        lambda e, o=t2_sb[tt % 2], r=bap, s2=sin_sb[:, sl]:
                        e.tensor_mul(o[:, :], r, s2),
                        t2waits, inc=True)
            bank_war[key] = [t2op]
            addop = g.op("dve",
                         lambda e, o=dstT[:, tt, sl], a=t1_sb[tt % 2], b=t2_sb[tt % 2]:
                         e.tensor_add(o, a[:, :], b[:, :]),
                         [t1op, t2op], inc=True)
            qt_war[bq] = [rop, t1op]
            t1_war[tt % 2] = addop
            t2_war[tt % 2] = addop
            rope_ready[(("q", "k")[wi], tt, qc)] = addop

        def emit_b2_st(st, key):
            """V projection tile st -> vt[:, st] ([V|32] layout)."""
            bap = bank_of[key]
            last = None
            for cb in range(2):
                fsl = slice(cb * 256, (cb + 1) * 256)
                first = True
                for ti, (xa, wa, wf) in enumerate(
                        b2_terms(4 <= st < 8, st >= 8)):
                    for pr in range(4):
                        waits = wf(pr)
                        if cb == 0 and first:
                            waits = waits + bank_war[key]
                            bank_war[key] = []
                        last = g.op("pe", mm(bap[:, fsl],
                                             xa[:, 2 * pr:2 * pr + 2,
                                                st * 128:(st + 1) * 128],
                                             wa[:, 2 * pr:2 * pr + 2, fsl],
                                             first, ti == 2 and pr == 3, DR),
                                    waits,
                                    inc=True if (cb == 1 and ti == 2 and pr == 3)
                                    else None)
                        first = False
            cop = g.op("act",
                       lambda e, o=vt[:, st, :, 0:64], i=bap:
                       e.copy(o, i.rearrange("p (h f) -> p h f", h=8)),
                       [last], inc=True)
            bank_war[key].append(cop)
            mset = g.op("dve",
                        lambda e, o=vt[:, st, :, 64:65]: e.memset(o, WS),
                        (), inc=True)
            vt_ready[st] = (cop, mset)

        # ---- B1 for qc=0 on all 8 banks (q->scp halves, k->px);
        #      rope deferred one unit so the DVE copy overlaps PE ----
        B1_KEYS = {(0, 0): "s00", (0, 1): "s01", (0, 2): "s10", (0, 3): "s11",
                   (1, 0): "px0", (1, 1): "px1", (1, 2): "px2", (1, 3): "px3"}
        prev_u2 = None
        for tt in range(4):
            for wi in range(2):
                u2 = emit_b1_cop(emit_b1_chain(0, wi, tt, B1_KEYS[(wi, tt)]))
                if prev_u2 is not None:
                    emit_b1_rope(prev_u2)
                prev_u2 = u2
        emit_b1_rope(prev_u2)

        # ---- slot-work queues threaded through phase C head slots ----
        sb_i = 0

        def next_slot_key():
            nonlocal sb_i
            key = f"px{2 + sb_i % 2}"
            sb_i += 1
            return key

        pending_units = []   # (qc, wi, tt)
        pending_rope = []    # cop states awaiting emit_b1_rope (1-slot delay)
        pending_b2 = []      # st
        pending_d = []       # (st, dc, qc)
        pending_n1 = []      # rb-broadcast DMA issues (1-slot delay)
        pending_n2 = []      # normalization multiplies (2-slot delay)

        esc_war = [[] for _ in range(N_ESC)]
        qc_mul = {}   # qc -> last normalization mul of that chunk (DVE order)
        av_war = [bank_war["px0"], bank_war["px1"]]
        bank_war["px0"] = []
        bank_war["px1"] = []
        prev_mul = None
        spi = 0
        epi = 0
        avj = 0
        oddj = 0
        outi = 0

        def emit_d_group(mid=None):
            nonlocal outi
            st, dc, dqc = pending_d.pop(0)
            i = outi
            outi += 1
            key = next_slot_key()
            bap = bank_of[key]
            extra = [("d_odd0", 32 * (dqc + 1)), ("d_odd1", 32 * (dqc + 1)),
                     D_WO]
            if dqc in qc_mul:
                extra.append(qc_mul[dqc])
            last = None
            for pp in range(4):
                waits = []
                if pp == 0:
                    waits = bank_war[key] + extra
                    bank_war[key] = []
                last = g.op("pe", mm(bap,
                                     attT[:, pp, st * 128:(st + 1) * 128],
                                     wo_sb[:, pp, dc * 512:(dc + 1) * 512],
                                     pp == 0, pp == 3),
                            waits, inc=True if pp == 3 else None)
                if pp == 1 and mid is not None:
                    mid()
            outsem = f"d_out{i % 2}"
            cwaits = [last]
            if i >= 2:
                cwaits.append((outsem, 16 * (i // 2)))
            cop = g.op("dve",
                       lambda e, o=osb[i % 2], b=bap:
                       e.tensor_copy(o[:, :], b),
                       cwaits, inc=True)
            bank_war[key].append(cop)
            dma("sp", out_d[st * 128:(st + 1) * 128, dc * 512:(dc + 1) * 512],
                osb[i % 2][:, :], outsem, [cop, (outsem, 16 * (i // 2))])

        # ---- B2 st0..7 upfront (st0..3 needed by qc0's AV; st4..7 keep the
        #      window-0 slots down to one B2 tenant each) ----
        for st in range(10):
            emit_b2_st(st, next_slot_key())
        pending_b2 = list(range(10, 16))
        pending_units = [(1, wi, tt) for tt in range(4) for wi in range(2)]

        # ---- phase C: per query chunk, 8 heads; slot work threaded in ----
        for qc in range(4):
            qsl = slice(qc * 512, (qc + 1) * 512)
            for slot_idx, h in enumerate((1, 0, 3, 2, 5, 4, 7, 6)):
                if pending_n1:
                    pending_n1.pop(0)()
                if len(pending_n2) >= (1 if qc == 3 and slot_idx >= 5 else 2):
                    pending_n2.pop(0)()

                p, half = h // 2, h % 2
                base = 64 * half
                n_kt = 4 * qc + 4
                n_pairs = 2 * qc + 2
                hp = 2 * qc + 1          # half (trimmed) diagonal pair
                avbank = px[avj % 2]
                ready = {}
                escbuf = {}

                def emit_score_pair(pa):
                    nonlocal spi, epi
                    sp_i = spi % 2
                    spi += 1
                    eb = epi % N_ESC
                    epi += 1
                    kt0 = 2 * pa
                    cs = slice(256, 512) if pa == hp else slice(0, 512)
                    qs = slice(qc * 512 + cs.start, qc * 512 + cs.stop)
                    s1 = g.op("pe", mm(scp[sp_i][:, 0, cs],
                                       kropeT[base:base + 64, p,
                                              kt0 * 128:(kt0 + 1) * 128],
                                       qropeT[base:base + 64, p, qs],
                                       True, True),
                              [rope_ready[("k", p, kt0 // 4)],
                               rope_ready[("q", p, qc)]] + bank_war[f"s{sp_i}0"],
                              inc=True)
                    bank_war[f"s{sp_i}0"] = []
                    s2 = g.op("pe", mm(scp[sp_i][:, 1, cs],
                                       kropeT[base:base + 64, p,
                                              (kt0 + 1) * 128:(kt0 + 2) * 128],
                                       qropeT[base:base + 64, p, qs],
                                       True, True),
                              [rope_ready[("k", p, (kt0 + 1) // 4)]]
                              + bank_war[f"s{sp_i}1"],
                              inc=True)
                    bank_war[f"s{sp_i}1"] = []
                    eop = g.op("act",
                               lambda e, o=esc_sb[eb], i=scp[sp_i], c=cs:
                               e.activation(o[:, :, c], i[:, :, c], EXP,
                                            bias=zeros_sb[:, 0:1],
                                            scale=EXPSCALE),
                               [s2, zeros_op] + esc_war[eb], inc=True)
                    esc_war[eb] = []
                    bank_war[f"s{sp_i}0"].append(eop)
                    bank_war[f"s{sp_i}1"].append(eop)
                    fin = eop
                    if pa >= 2 * qc:  # diagonal pair: triangular fill
                        acs = slice(0, 256) if pa == 2 * qc else slice(256, 512)
                        fin = g.op("gp",
                                   lambda e, o=esc_sb[eb], c=acs:
                                   e.affine_select(out=o[:, :, c], in_=o[:, :, c],
                                                   pattern=[[-128, 2], [1, 256]],
                                                   compare_op=mybir.AluOpType.is_ge,
                                                   fill=0.0, base=0,
                                                   channel_multiplier=-1),
                                   [eop], inc=True)
                    ready[pa] = fin
                    escbuf[pa] = eb

                def emit_av_pair(pa):
                    nonlocal last_av
                    eb = escbuf[pa]
                    cs = slice(256, 512) if pa == hp else slice(0, 512)
                    for hi in range(2):
                        kt = 2 * pa + hi
                        waits = ([ready[pa]] if hi == 0 else []) \
                            + [vt_ready[kt][0], vt_ready[kt][1]]
                        if kt == 0:
                            waits = waits + av_war[avj % 2]
                            av_war[avj % 2] = []
                        op = g.op("pe", mm(avbank[0:65, cs], vt[:, kt, h, :],
                                           esc_sb[eb][:, hi, cs],
                                           kt == 0, kt == n_kt - 1),
                                  waits, inc=True if kt == n_kt - 1 else None)
                        last_av = op
                    esc_war[eb] = [last_av]

                last_av = None
                for pa in range(min(2, n_pairs)):
                    emit_score_pair(pa)
                # threaded PE work sits between the scores and the AVs so the
                # exp+mask latency of the first pairs is hidden behind it, and
                # the rope/chain PE ops trail the DVE results they consume
                if pending_rope:
                    emit_b1_rope(pending_rope.pop(0))
                if pending_b2:
                    emit_b2_st(pending_b2.pop(0), next_slot_key())
                # defer half of qc3's units into window 3, which otherwise
                # has no threaded PE work to cover the scalar engine's exps
                has_unit = bool(pending_units) and not (qc == 2 and slot_idx >= 4)
                if has_unit:
                    if pending_d and slot_idx >= 1:
                        emit_d_group()
                    if n_pairs > 2:
                        emit_score_pair(2)
                    uqc, uwi, utt = pending_units.pop(0)
                    pending_rope.append(emit_b1_cop(
                        emit_b1_chain(uqc, uwi, utt, next_slot_key())))
                elif pending_d and slot_idx >= 1 and n_pairs > 2:
                    # no chain this slot: spread the out-projection group's
                    # matmuls around sc2 so neither sc2 nor sc3 outruns exp
                    emit_d_group(mid=lambda: emit_score_pair(2))
                else:
                    if n_pairs > 2:
                        emit_score_pair(2)
                    if pending_d and slot_idx >= 1:
                        emit_d_group()
                for pa in range(3, min(LOOKAHEAD, n_pairs)):
                    emit_score_pair(pa)
                nxtp = LOOKAHEAD
                for pa in range(n_pairs):
                    emit_av_pair(pa)
                    if nxtp < n_pairs:
                        emit_score_pair(nxtp)
                        nxtp += 1

                # normalization pipeline: reciprocal now; rb-broadcast DMA one
                # slot later; multiply (+ odd-half DMA) two slots later, so
                # the DMA latency never blocks the in-order DVE queue.
                myavj = avj
                rbsem = f"d_rb{myavj % 2}"
                rwaits = [last_av]
                if myavj >= 2:
                    rwaits.append((rbsem, 16 * (myavj // 2)))
                rop = g.op("dve",
                           lambda e, o=rcp_sb[myavj % 2], i=avbank:
                           e.reciprocal(o[64:65, :], i[64:65, :]),
                           rwaits, inc=True)

                def norm_rb(rop=rop, myavj=myavj, rbsem=rbsem):
                    rsrc = rcp_sb[myavj % 2][64:65, :]
                    bcast = bass.AP(tensor=rsrc.tensor, offset=rsrc.offset,
                                    ap=[rsrc.ap[0], [0, 64], rsrc.ap[1]])
                    dma("sp", rb_sb[myavj % 2][:, :], bcast, rbsem,
                        [rop, (rbsem, 16 * (myavj // 2))])

                def norm_mul(myavj=myavj, rbsem=rbsem, avbank=avbank,
                             p=p, half=half, qsl=qsl, myqc=qc):
                    nonlocal prev_mul, oddj
                    mwaits = [(rbsem, 16 * (myavj // 2 + 1))]
                    if prev_mul is not None:
                        mwaits.append(prev_mul)
                    if half == 0:
                        dst = attT[0:64, p, qsl]
                    else:
                        oddsem = f"d_odd{oddj % 2}"
                        if oddj >= 2:
                            mwaits.append((oddsem, 16 * (oddj // 2)))
                        dst = odd_sb[oddj % 2][:, :]
                    mop = g.op("dve",
                               lambda e, o=dst, a=avbank, r=rb_sb[myavj % 2]:
                               e.tensor_mul(o, a[0:64, :], r[:, :]),
                               mwaits, inc=True)
                    prev_mul = mop
                    qc_mul[myqc] = mop
                    if half == 1:
                        dma("sp", attT[64:128, p, qsl], odd_sb[oddj % 2][:, :],
                            oddsem, [mop, (oddsem, 16 * (oddj // 2))])
                        oddj += 1
                    av_war[myavj % 2] = [mop]

                pending_n1.append(norm_rb)
                pending_n2.append(norm_mul)
                avj += 1

                if qc == 3 and slot_idx >= 4 and pending_n1:
                    pending_n1.pop(0)()

            if qc + 2 <= 3:
                pending_units.extend(
                    (qc + 2, wi, tt) for tt in range(4) for wi in range(2))

            for st in range(4 * qc, 4 * qc + 4):
                for dc in range(2):
                    pending_d.append((st, dc, qc))

        while pending_n1:
            pending_n1.pop(0)()
        while pending_n2:
            pending_n2.pop(0)()
        while pending_rope:
            emit_b1_rope(pending_rope.pop(0))
        while pending_d:
            emit_d_group()

        g.resolve()

        with nc.allow_low_precision(reason="fp8/bf16 attention intermediates"), \
                nc.Block() as block:
            @block.tensor
            def _(eng):
                g.emit("pe", eng, sems)

            @block.scalar
            def _(eng):
                g.emit("act", eng, sems)

            @block.vector
            def _(eng):
                g.emit("dve", eng, sems)

            @block.gpsimd
            def _(eng):
                g.emit("gp", eng, sems)

            @block.sync
            def _(eng):
                g.emit("sp", eng, sems)

    return nc


def _get_nc():
    global _nc_cache
    if _nc_cache is None:
        _nc_cache = _build_nc()
    return _nc_cache


def _host_consts():
    perm = np.concatenate([
        h * HD + np.concatenate([np.arange(0, HD, 2), np.arange(1, HD, 2)])
        for h in range(8)
    ])
    P2 = np.zeros((128, 128), np.float32)
    for b0 in (0, 64):
        P2[np.arange(b0, b0 + 32), np.arange(b0 + 32, b0 + 64)] = 1.0
        P2[np.arange(b0 + 32, b0 + 64), np.arange(b0, b0 + 32)] = 1.0
    return perm, P2.astype(NPBF16)


def _host_tables(cos, sin):
    """cos/sin tables tiled to 128 partitions; the sin table carries the P2
    rotation's signs (-,+ per 32-row block) so the on-chip rotation is a
    pure partition-block swap."""
    cosr = np.ascontiguousarray(np.tile(cos.T, (4, 1))).astype(NPBF16)
    sgn = np.concatenate([-np.ones(32, np.float32), np.ones(32, np.float32)]
                         * 2)[:, None]
    sinr = np.ascontiguousarray(np.tile(sin.T, (4, 1)) * sgn).astype(NPBF16)
    return cosr, sinr


def _split8(a):
    """f32 -> (hi, lo) fp8e4m3 with lo = fp8(a - hi)."""
    hi = np.ascontiguousarray(a).astype(NPFP8)
    lo = (a - hi.astype(np.float32)).astype(NPFP8)
    return hi, lo


def kernel(x, freqs_cos, freqs_sin, wq, wk, wv, wo):
    global last_results
    x = np.asarray(x, np.float32)
    cos = np.asarray(freqs_cos, np.float32)
    sin = np.asarray(freqs_sin, np.float32)
    wq = np.asarray(wq, np.float32)
    wk = np.asarray(wk, np.float32)
    wv = np.asarray(wv, np.float32)
    wo = np.asarray(wo, np.float32)

    perm, protT = _host_consts()
    cosr, sinr = _host_tables(cos, sin)

    xs = [_split8(np.ascontiguousarray(x[b].T)) for b in range(B)]
    wqs, wks, wvs, wos = {}, {}, {}, {}
    for gg in range(2):
        gsl = slice(gg * HG, (gg + 1) * HG)
        wqs[gg] = _split8(np.ascontiguousarray((wq[gsl][perm] * WS).T))
        wks[gg] = _split8(np.ascontiguousarray((wk[gsl][perm] * WS).T))
        wvs[gg] = _split8(np.ascontiguousarray((wv[gsl] * WS).T))
        wos[gg] = np.ascontiguousarray(wo.T[gsl]).astype(NPBF16)

    in_maps = []
    for c in range(N_CORES):
        b, gg = c // 2, c % 2
        in_maps.append({
            "xh": xs[b][0], "xl": xs[b][1],
            "wqh": wqs[gg][0], "wql": wqs[gg][1],
            "wkh": wks[gg][0], "wkl": wks[gg][1],
            "wvh": wvs[gg][0], "wvl": wvs[gg][1],
            "woT": wos[gg],
            "cosr": cosr,
            "sinr": sinr,
            "protT": protT,
        })

    nc = _get_nc()
    last_results = run_bass_kernel_spmd(nc, in_maps, list(range(N_CORES)))
    res = last_results.results

    out = np.empty((B, S, D), np.float32)
    for b in range(B):
        out[b] = res[2 * b]["out"] + res[2 * b + 1]["out"]
    return out


# revision 86
# speedup vs baseline: 1.6575x; 1.0006x over previous
"""Causal multi-head attention (B=4, S=2048, D=1024, H=16, HD=64) with RoPE,
distributed over 8 TRN2 NeuronCores as (batch x head-group): core c handles
batch c//2 and heads (c%2)*8..(c%2)*8+7.  Each core computes a [2048, 1024]
partial of out@wo.T restricted to its 8 heads; the host sums the two partials
per batch.

Written in raw Bass (explicit per-engine programs + manual semaphores): the
walrus build in this container rejects instructions carrying more than one
attached sync command, which rules out TileContext; all waits here are
standalone wait_ge instructions.

v2 changes over the baseline (440.9us -> 266.1us on the CoreSim cost model):
  - Q/K/V projections run as fp8e4m3 DoubleRow matmuls (0.5 cycles/row, 2
    contraction k-tiles per instruction = 4x bf16 throughput).  The host
    splits x and the 32x-scaled weights into (hi, lo) fp8 pairs; each
    projection is the 3-term sum x_hi*w_hi + x_lo*w_hi + x_hi*w_lo
    accumulated in one PSUM group, which matches bf16 accuracy (the 32x
    scale keeps the small weights out of fp8's denormal range).  The scale
    is compensated by the exp scale (scores carry 32*32) and by setting
    the [V|ones] ones row to 32 so the softmax denominator cancels V's.
    Scores/AV/out-projection stay bf16: quantizing q/k/att to fp8 puts
    ~2.5% multiplicative noise straight on the output and fails the gate.
  - Causal trimming: the second diagonal key-pair of each query chunk only
    computes/exps/masks query columns [256:512); the first diagonal pair
    masks only columns [0:256).
  - One exp instruction per score pair ([128,2,512] in a single AP), 6 esc
    buffers, score lookahead 4.
  - Software-pipelined slot schedule: each head-slot of phase C threads in
    B1 rope-projection chains for qc+1, B2 V-projection tiles, and the
    previous chunk's output-projection groups on shared px2/px3 banks, so
    PE keeps running while the scalar engine exps.  The B1 psum->sbuf
    copy, rotation, and rope combine are deferred one slot behind the
    chain; the normalization runs as a 3-stage pipeline (reciprocal ->
    rb-broadcast DMA one slot later -> multiply two slots later) so DMA
    latency never blocks the in-order DVE queue.  Odd-half heads run
    first so the last head's attT write is the direct (non-DMA) half.
  - Input DMAs spread kt-pair-major across the SP/Act/Pool queues with
    the x tiles split into sequence halves, separately semaphored, so the
    first projection chains only gate on the data they read.
"""

import sys

if "/opt/trn_rl_repo" not in sys.path:
    sys.path.insert(0, "/opt/trn_rl_repo")

from contextlib import ExitStack

import numpy as np
import ml_dtypes

import concourse.bass as bass
from concourse import mybir
from concourse.bass_utils import run_bass_kernel_spmd

BF16 = mybir.dt.bfloat16
FP8 = mybir.dt.float8e4
F32 = mybir.dt.float32
NPBF16 = ml_dtypes.bfloat16
NPFP8 = ml_dtypes.float8_e4m3
EXP = mybir.ActivationFunctionType.Exp
DR = mybir.MatmulPerfMode.DoubleRow

B, S, D, H, HD = 4, 2048, 1024, 16, 64
HG = 512
N_CORES = 8
WS = 32.0                       # fp8 weight pre-scale
EXPSCALE = 0.125 / (WS * WS)    # 1/8192, exact in f32
N_ESC = 6
LOOKAHEAD = 4

_nc_cache = None
last_results = None


class _Op:
    __slots__ = ("eng", "fn", "waits", "inc", "done")

    def __init__(self, eng, fn, waits, inc):
        self.eng, self.fn, self.waits, self.inc = eng, fn, list(waits), inc
        self.done = None  # (sem_name, value) proving completion


class _Gen:
    """Pass-1 op recorder; resolves symbolic op-completion waits to semaphore
    counts, then replays each engine's program inside its Block closure."""

    ENGS = ("pe", "act", "dve", "gp", "sp")

    def __init__(self):
        self.ops = {e: [] for e in self.ENGS}

    def op(self, eng, fn, waits=(), inc=None):
        o = _Op(eng, fn, waits, inc)
        self.ops[eng].append(o)
        return o

    def resolve(self):
        for eng in self.ENGS:
            sem = "s_" + eng
            cum = 0
            cums = {}
            for o in self.ops[eng]:
                if o.inc is True:
                    cum += 1
                    o.done = (sem, cum)
                elif o.inc is not None:  # DMA: (dma_sem, 16)
                    sn, amt = o.inc
                    cums[sn] = cums.get(sn, 0) + amt
                    o.done = (sn, cums[sn])
            carry = None
            for o in reversed(self.ops[eng]):
                if o.inc is True:
                    carry = o.done
                elif o.inc is None and carry is not None:
                    o.done = carry

    def emit(self, eng_name, eng_obj, sems):
        observed = {}
        for o in self.ops[eng_name]:
            todo = {}
            for w in o.waits:
                semn, val = w.done if isinstance(w, _Op) else (w[0], w[1])
                if val > todo.get(semn, 0):
                    todo[semn] = val
            for semn, val in todo.items():
                if observed.get(semn, 0) < val:
                    eng_obj.wait_ge(sems[semn], val)
                    observed[semn] = val
            inst = o.fn(eng_obj)
            if o.inc is not None and o.inc is not True:
                inst.then_inc(sems[o.inc[0]], o.inc[1])
            elif o.inc is True:
                inst.then_inc(sems["s_" + eng_name], 1)


def _build_nc():
    nc = bass.Bass()

    xh_d = nc.declare_dram_parameter("xh", [D, S], FP8, isOutput=False)
    xl_d = nc.declare_dram_parameter("xl", [D, S], FP8, isOutput=False)
    wqh_d = nc.declare_dram_parameter("wqh", [D, HG], FP8, isOutput=False)
    wql_d = nc.declare_dram_parameter("wql", [D, HG], FP8, isOutput=False)
    wkh_d = nc.declare_dram_parameter("wkh", [D, HG], FP8, isOutput=False)
    wkl_d = nc.declare_dram_parameter("wkl", [D, HG], FP8, isOutput=False)
    wvh_d = nc.declare_dram_parameter("wvh", [D, HG], FP8, isOutput=False)
    wvl_d = nc.declare_dram_parameter("wvl", [D, HG], FP8, isOutput=False)
    wo_d = nc.declare_dram_parameter("woT", [HG, D], BF16, isOutput=False)
    cos_d = nc.declare_dram_parameter("cosr", [128, S], BF16, isOutput=False)
    sin_d = nc.declare_dram_parameter("sinr", [128, S], BF16, isOutput=False)
    prot_d = nc.declare_dram_parameter("protT", [128, 128], BF16, isOutput=False)
    out_d = nc.declare_dram_parameter("out", [S, D], F32, isOutput=True)

    sem_names = (["s_pe", "s_act", "s_dve", "s_gp", "s_sp"]
                 + [f"d_kh{k}" for k in range(8)]
                 + [f"d_khc{k}" for k in range(8)]
                 + [f"d_xhb{k}" for k in range(8)]
                 + [f"d_xlc{k}" for k in range(8)]
                 + [f"d_xl{k}" for k in range(8)]
                 + [f"d_xlb{k}" for k in range(8)]
                 + [f"d_wl{k}" for k in range(8)]
                 + ["d_wvh", "d_wvl", "d_cs", "d_pcs", "d_wo", "d_rb0", "d_rb1",
                    "d_odd0", "d_odd1", "d_out0", "d_out1"])

    with ExitStack() as ctx:
        sb = lambda name, shape, dt: ctx.enter_context(nc.sbuf_tensor(name, shape, dt))

        xh = sb("xh_sb", [128, 8, S], FP8)
        xl = sb("xl_sb", [128, 8, S], FP8)
        wqh = sb("wqh_sb", [128, 8, HG], FP8)
        wql = sb("wql_sb", [128, 8, HG], FP8)
        wkh = sb("wkh_sb", [128, 8, HG], FP8)
        wkl = sb("wkl_sb", [128, 8, HG], FP8)
        wvh = sb("wvh_sb", [128, 8, HG], FP8)
        wvl = sb("wvl_sb", [128, 8, HG], FP8)
        wo_sb = sb("wo_sb", [128, 4, D], BF16)
        cos_sb = sb("cos_sb", [128, S], BF16)
        sin_sb = sb("sin_sb", [128, S], BF16)
        prot_sb = sb("prot_sb", [128, 128], BF16)
        qropeT = sb("qropeT", [128, 4, S], BF16)
        kropeT = sb("kropeT", [128, 4, S], BF16)
        vt = sb("vt", [128, 16, 8, 65], BF16)
        attT = sb("attT", [128, 4, S], BF16)
        zeros_sb = sb("zeros_sb", [128, 1], F32)
        qt_sb = [sb(f"qt_sb{i}", [128, 512], BF16) for i in range(3)]
        t1_sb = [sb(f"t1_sb{i}", [128, 512], BF16) for i in range(2)]
        t2_sb = [sb(f"t2_sb{i}", [128, 512], BF16) for i in range(2)]
        esc_sb = [sb(f"esc_sb{i}", [128, 2, 512], BF16) for i in range(N_ESC)]
        rcp_sb = [sb(f"rcp_sb{i}", [65, 512], BF16) for i in range(2)]
        rb_sb = [sb(f"rb_sb{i}", [64, 512], BF16) for i in range(2)]
        odd_sb = [sb(f"odd_sb{i}", [64, 512], BF16) for i in range(2)]
        osb = [sb(f"osb{i}", [128, 512], F32) for i in range(2)]

        scp = [ctx.enter_context(nc.psum_tensor(f"scp{i}", [128, 2, 512], F32))
               for i in range(2)]
        px = [ctx.enter_context(nc.psum_tensor(f"px{i}", [128, 512], F32))
              for i in range(4)]

        sems = {n: ctx.enter_context(nc.semaphore(n)) for n in sem_names}

        g = _Gen()

        def dma(eng, dst, src, sem, waits=()):
            return g.op(eng,
                        lambda e, a=dst, b=src: e.dma_start(out=a, in_=b),
                        waits, inc=(sem, 16))

        # ---- input DMAs, spread over all four DMA-capable queues so the
        #      first B1 chain's kt pairs arrive as fast as possible; rope
        #      tables front-loaded so the first rot/rope ops don't wait ----
        ksl_ = lambda kt: slice(kt * 128, (kt + 1) * 128)
        SQ, SC = slice(0, S // 4), slice(S // 4, S // 2)
        SB = slice(S // 2, S)
        # kt-pair-major so the first B1 chain can chase the streams; x tiles
        # split into sequence halves (the early chains only touch cols<1024):
        #   sp:  xh even + wqh         act: xh odd + wkh
        #   gp:  xl even + wql/wkl even  (act gets the odd xl/w-lo later)
        for pr in range(4):
            k0, k1 = 2 * pr, 2 * pr + 1
            dma("sp", xh[:, k0, SQ], xh_d[ksl_(k0), SQ], f"d_kh{k0}")
            dma("sp", wqh[:, k0, :], wqh_d[ksl_(k0), :], f"d_kh{k0}")
            dma("sp", wqh[:, k1, :], wqh_d[ksl_(k1), :], f"d_kh{k1}")
            dma("act", wkh[:, k0, :], wkh_d[ksl_(k0), :], f"d_kh{k0}")
            dma("act", xh[:, k1, SQ], xh_d[ksl_(k1), SQ], f"d_kh{k1}")
            dma("act", wkh[:, k1, :], wkh_d[ksl_(k1), :], f"d_kh{k1}")
            dma("gp", xl[:, k0, SQ], xl_d[ksl_(k0), SQ], f"d_xl{k0}")
        for kt in range(1, 8, 2):
            dma("act", xl[:, kt, SQ], xl_d[ksl_(kt), SQ], f"d_xl{kt}")
        for kt in range(8):
            klq = "gp" if kt % 2 == 0 else "act"
            dma(klq, wql[:, kt, :], wql_d[ksl_(kt), :], f"d_wl{kt}")
            dma(klq, wkl[:, kt, :], wkl_d[ksl_(kt), :], f"d_wl{kt}")
        for kt in range(8):
            xq = "sp" if kt % 2 == 0 else "act"
            dma(xq, xh[:, kt, SC], xh_d[ksl_(kt), SC], f"d_khc{kt}")
            dma(xq if kt % 2 else "gp", xl[:, kt, SC], xl_d[ksl_(kt), SC],
                f"d_xlc{kt}")
        dma("sp", prot_sb[:, :], prot_d[:, :], "d_pcs")
        dma("gp", cos_sb[:, :], cos_d[:, :], "d_cs")
        dma("gp", sin_sb[:, :], sin_d[:, :], "d_cs")
        for kt in range(8):
            xq = "sp" if kt % 2 == 0 else "act"
            dma(xq, xh[:, kt, SB], xh_d[ksl_(kt), SB], f"d_xhb{kt}")
            dma(xq if kt % 2 else "gp", xl[:, kt, SB], xl_d[ksl_(kt), SB],
                f"d_xlb{kt}")
        for kt in range(8):
            dma("act", wvh[:, kt, :], wvh_d[ksl_(kt), :], "d_wvh")
            dma("gp", wvl[:, kt, :], wvl_d[ksl_(kt), :], "d_wvl")
        for p in range(4):
            dma("gp", wo_sb[:, p, :], wo_d[p * 128:(p + 1) * 128, :], "d_wo")
        D_PCS = ("d_pcs", 16)
        D_COS = ("d_cs", 32)
        D_SIN = ("d_cs", 32)
        D_WO = ("d_wo", 16 * 4)
        D_WVH = ("d_wvh", 16 * 8)
        D_WVL = ("d_wvl", 16 * 8)

        def khw(pr, c=False, b=False):
            w = [(f"d_kh{2 * pr}", 48), (f"d_kh{2 * pr + 1}", 48)]
            if c:
                w += [(f"d_khc{2 * pr}", 16), (f"d_khc{2 * pr + 1}", 16)]
            if b:
                w += [(f"d_xhb{2 * pr}", 16), (f"d_xhb{2 * pr + 1}", 16)]
            return w

        def xlw(pr, c=False, b=False):
            w = [(f"d_xl{2 * pr}", 16), (f"d_xl{2 * pr + 1}", 16)]
            if c:
                w += [(f"d_xlc{2 * pr}", 16), (f"d_xlc{2 * pr + 1}", 16)]
            if b:
                w += [(f"d_xlb{2 * pr}", 16), (f"d_xlb{2 * pr + 1}", 16)]
            return w

        def wlw(pr):
            return [(f"d_wl{2 * pr}", 32), (f"d_wl{2 * pr + 1}", 32)]

        def mm(bank_ap, lhsT, rhs, start, stop, pm=None):
            return lambda e, o=bank_ap, l=lhsT, r=rhs, s=start, t=stop, m=pm: \
                e.matmul(o, lhsT=l, rhs=r, start=s, stop=t,
                         perf_mode=m, skip_group_check=True)

        zeros_op = g.op("dve", lambda e: e.memset(zeros_sb[:, :], 0.0), (), inc=True)

        # 8 psum banks: scp halves s00,s01,s10,s11 + px0..3
        bank_of = {"s00": scp[0][:, 0, :], "s01": scp[0][:, 1, :],
                   "s10": scp[1][:, 0, :], "s11": scp[1][:, 1, :],
                   "px0": px[0][:, :], "px1": px[1][:, :],
                   "px2": px[2][:, :], "px3": px[3][:, :]}
        bank_war = {k: [] for k in bank_of}
        qt_war = [[] for _ in range(3)]
        t1_war = [None, None]
        t2_war = [None, None]
        rope_ready = {}
        qtbuf = 0
        vt_ready = {}

        # projection DR term table: (x hi/lo, w hi/lo, extra dma waits fn);
        # `b` asks for the second sequence half of the x tiles (cols >= 1024)
        def b1_terms(wi, c, b):
            wh = (wqh, wkh)[wi]
            wl = (wql, wkl)[wi]
            return [(xh, wh, lambda pr: khw(pr, c, b)),
                    (xl, wh, lambda pr: khw(pr) + xlw(pr, c, b)),
                    (xh, wl, lambda pr: khw(pr, c, b) + wlw(pr))]

        def b2_terms(c, b):
            return [(xh, wvh, lambda pr: khw(pr, c, b) + [D_WVH]),
                    (xl, wvh, lambda pr: xlw(pr, c, b) + [D_WVH]),
                    (xh, wvl, lambda pr: khw(pr, c, b) + [D_WVL])]

        def emit_b1_chain(qc, wi, tt, key):
            """12-instr DR chain x2 (col halves) -> psum bank `key` for
            (q|k, tt) of query chunk qc.  Returns state for emit_b1_rope."""
            bap = bank_of[key]
            terms = b1_terms(wi, qc == 1, qc >= 2)
            last = None
            for cb in range(2):
                csl = slice(qc * 512 + cb * 256, qc * 512 + (cb + 1) * 256)
                first = True
                for ti, (xa, wa, wf) in enumerate(terms):
                    for pr in range(4):
                        waits = wf(pr)
                        if cb == 0 and first:
                            waits = waits + bank_war[key]
                            bank_war[key] = []
                        last = g.op("pe", mm(bap[:, cb * 256:(cb + 1) * 256],
                                             wa[:, 2 * pr:2 * pr + 2,
                                                tt * 128:(tt + 1) * 128],
                                             xa[:, 2 * pr:2 * pr + 2, csl],
                                             first, ti == 2 and pr == 3, DR),
                                    waits,
                                    inc=True if (cb == 1 and ti == 2 and pr == 3)
                                    else None)
                        first = False
            return (qc, wi, tt, key, last)

        def emit_b1_cop(u):
            """psum->sbuf copy of a finished B1 chain (DVE; emitted right
            after the chain so DVE starts it while PE moves on)."""
            nonlocal qtbuf
            qc, wi, tt, key, last = u
            bap = bank_of[key]
            bq = qtbuf % 3
            qtbuf += 1
            cop = g.op("dve",
                       lambda e, a=qt_sb[bq], b=bap:
                       e.tensor_copy(a[:, :], b),
                       [last] + qt_war[bq], inc=True)
            qt_war[bq] = []
            return (qc, wi, tt, key, cop, bq)

        def emit_b1_rope(u2):
            """P2 rotation + cos/sin rope combine, one slot after the copy
            (rotation signs live in the host-negated sin table)."""
            qc, wi, tt, key, cop, bq = u2
            bap = bank_of[key]
            dstT = (qropeT, kropeT)[wi]
            sl = slice(qc * 512, (qc + 1) * 512)
            rop = g.op("pe", mm(bap, prot_sb[:, :], qt_sb[bq][:, :], True, True),
                       [cop, D_PCS], inc=True)
            t1waits = [cop, D_COS]
            if t1_war[tt % 2] is not None:
                t1waits.append(t1_war[tt % 2])
            t1op = g.op("dve",
                        lambda e, o=t1_sb[tt % 2], a=qt_sb[bq], c=cos_sb[:, sl]:
                        e.tensor_mul(o[:, :], a[:, :], c),
                        t1waits, inc=True)
            t2waits = [rop, D_SIN]
            if t2_war[tt % 2] is not None:
                t2waits.append(t2_war[tt % 2])
            t2op = g.op("dve",
                        lambda e, o=t2_sb[tt % 2], r=bap, s2=sin_sb[:, sl]:
                        e.tensor_mul(o[:, :], r, s2),
                        t2waits, inc=True)
            bank_war[key] = [t2op]
            addop = g.op("dve",
                         lambda e, o=dstT[:, tt, sl], a=t1_sb[tt % 2], b=t2_sb[tt % 2]:
                         e.tensor_add(o, a[:, :], b[:, :]),
                         [t1op, t2op], inc=True)
            qt_war[bq] = [rop, t1op]
            t1_war[tt % 2] = addop
            t2_war[tt % 2] = addop
            rope_ready[(("q", "k")[wi], tt, qc)] = addop

        def emit_b2_st(st, key):
            """V projection tile st -> vt[:, st] ([V|32] layout)."""
            bap = bank_of[key]
            last = None
            for cb in range(2):
                fsl = slice(cb * 256, (cb + 1) * 256)
                first = True
                for ti, (xa, wa, wf) in enumerate(
                        b2_terms(4 <= st < 8, st >= 8)):
                    for pr in range(4):
                        waits = wf(pr)
                        if cb == 0 and first:
                            waits = waits + bank_war[key]
                            bank_war[key] = []
                        last = g.op("pe", mm(bap[:, fsl],
                                             xa[:, 2 * pr:2 * pr + 2,
                                                st * 128:(st + 1) * 128],
                                             wa[:, 2 * pr:2 * pr + 2, fsl],
                                             first, ti == 2 and pr == 3, DR),
                                    waits,
                                    inc=True if (cb == 1 and ti == 2 and pr == 3)
                                    else None)
                        first = False
            cop = g.op("act",
                       lambda e, o=vt[:, st, :, 0:64], i=bap:
                       e.copy(o, i.rearrange("p (h f) -> p h f", h=8)),
                       [last], inc=True)
            bank_war[key].append(cop)
            mset = g.op("dve",
                        lambda e, o=vt[:, st, :, 64:65]: e.memset(o, WS),
                        (), inc=True)
            vt_ready[st] = (cop, mset)

        # ---- B1 for qc=0 on all 8 banks (q->scp halves, k->px);
        #      rope deferred one unit so the DVE copy overlaps PE ----
        B1_KEYS = {(0, 0): "s00", (0, 1): "s01", (0, 2): "s10", (0, 3): "s11",
                   (1, 0): "px0", (1, 1): "px1", (1, 2): "px2", (1, 3): "px3"}
        prev_u2 = None
        for tt in range(4):
            for wi in range(2):
                u2 = emit_b1_cop(emit_b1_chain(0, wi, tt, B1_KEYS[(wi, tt)]))
                if prev_u2 is not None:
                    emit_b1_rope(prev_u2)
                prev_u2 = u2
        emit_b1_rope(prev_u2)

        # ---- slot-work queues threaded through phase C head slots ----
        sb_i = 0

        def next_slot_key():
            nonlocal sb_i
            key = f"px{2 + sb_i % 2}"
            sb_i += 1
            return key

        pending_units = []   # (qc, wi, tt)
        pending_rope = []    # cop states awaiting emit_b1_rope (1-slot delay)
        pending_b2 = []      # st
        pending_d = []       # (st, dc, qc)
        pending_n1 = []      # rb-broadcast DMA issues (1-slot delay)
        pending_n2 = []      # normalization multiplies (2-slot delay)

        esc_war = [[] for _ in range(N_ESC)]
        qc_mul = {}   # qc -> last normalization mul of that chunk (DVE order)
        av_war = [bank_war["px0"], bank_war["px1"]]
        bank_war["px0"] = []
        bank_war["px1"] = []
        prev_mul = None
        spi = 0
        epi = 0
        avj = 0
        oddj = 0
        outi = 0

        def emit_d_group(mid=None):
            nonlocal outi
            st, dc, dqc = pending_d.pop(0)
            i = outi
            outi += 1
            key = next_slot_key()
            bap = bank_of[key]
            extra = [("d_odd0", 32 * (dqc + 1)), ("d_odd1", 32 * (dqc + 1)),
                     D_WO]
            if dqc in qc_mul:
                extra.append(qc_mul[dqc])
            last = None
            for pp in range(4):
                waits = []
                if pp == 0:
                    waits = bank_war[key] + extra
                    bank_war[key] = []
                last = g.op("pe", mm(bap,
                                     attT[:, pp, st * 128:(st + 1) * 128],
                                     wo_sb[:, pp, dc * 512:(dc + 1) * 512],
                                     pp == 0, pp == 3),
                            waits, inc=True if pp == 3 else None)
                if pp == 1 and mid is not None:
                    mid()
            outsem = f"d_out{i % 2}"
            cwaits = [last]
            if i >= 2:
                cwaits.append((outsem, 16 * (i // 2)))
            cop = g.op("dve",
                       lambda e, o=osb[i % 2], b=bap:
                       e.tensor_copy(o[:, :], b),
                       cwaits, inc=True)
            bank_war[key].append(cop)
            dma("sp", out_d[st * 128:(st + 1) * 128, dc * 512:(dc + 1) * 512],
                osb[i % 2][:, :], outsem, [cop, (outsem, 16 * (i // 2))])

        # ---- B2 st0..7 upfront (st0..3 needed by qc0's AV; st4..7 keep the
        #      window-0 slots down to one B2 tenant each) ----
        for st in range(12):
            emit_b2_st(st, next_slot_key())
        pending_b2 = list(range(12, 16))
        pending_units = [(1, wi, tt) for tt in range(4) for wi in range(2)]

        # ---- phase C: per query chunk, 8 heads; slot work threaded in ----
        for qc in range(4):
            qsl = slice(qc * 512, (qc + 1) * 512)
            for slot_idx, h in enumerate((1, 0, 3, 2, 5, 4, 7, 6)):
                if pending_n1:
                    pending_n1.pop(0)()
                if len(pending_n2) >= (1 if qc == 3 and slot_idx >= 5 else 2):
                    pending_n2.pop(0)()

                p, half = h // 2, h % 2
                base = 64 * half
                n_kt = 4 * qc + 4
                n_pairs = 2 * qc + 2
                hp = 2 * qc + 1          # half (trimmed) diagonal pair
                avbank = px[avj % 2]
                ready = {}
                escbuf = {}

                def emit_score_pair(pa):
                    nonlocal spi, epi
                    sp_i = spi % 2
                    spi += 1
                    eb = epi % N_ESC
                    epi += 1
                    kt0 = 2 * pa
                    cs = slice(256, 512) if pa == hp else slice(0, 512)
                    qs = slice(qc * 512 + cs.start, qc * 512 + cs.stop)
                    s1 = g.op("pe", mm(scp[sp_i][:, 0, cs],
                                       kropeT[base:base + 64, p,
                                              kt0 * 128:(kt0 + 1) * 128],
                                       qropeT[base:base + 64, p, qs],
                                       True, True),
                              [rope_ready[("k", p, kt0 // 4)],
                               rope_ready[("q", p, qc)]] + bank_war[f"s{sp_i}0"],
                              inc=True)
                    bank_war[f"s{sp_i}0"] = []
                    s2 = g.op("pe", mm(scp[sp_i][:, 1, cs],
                                       kropeT[base:base + 64, p,
                                              (kt0 + 1) * 128:(kt0 + 2) * 128],
                                       qropeT[base:base + 64, p, qs],
                                       True, True),
                              [rope_ready[("k", p, (kt0 + 1) // 4)]]
                              + bank_war[f"s{sp_i}1"],
                              inc=True)
                    bank_war[f"s{sp_i}1"] = []
                    eop = g.op("act",
                               lambda e, o=esc_sb[eb], i=scp[sp_i], c=cs:
                               e.activation(o[:, :, c], i[:, :, c], EXP,
                                            bias=zeros_sb[:, 0:1],
                                            scale=EXPSCALE),
                               [s2, zeros_op] + esc_war[eb], inc=True)
                    esc_war[eb] = []
                    bank_war[f"s{sp_i}0"].append(eop)
                    bank_war[f"s{sp_i}1"].append(eop)
                    fin = eop
                    if pa >= 2 * qc:  # diagonal pair: triangular fill
                        acs = slice(0, 256) if pa == 2 * qc else slice(256, 512)
                        fin = g.op("gp",
                                   lambda e, o=esc_sb[eb], c=acs:
                                   e.affine_select(out=o[:, :, c], in_=o[:, :, c],
                                                   pattern=[[-128, 2], [1, 256]],
                                                   compare_op=mybir.AluOpType.is_ge,
                                                   fill=0.0, base=0,
                                                   channel_multiplier=-1),
                                   [eop], inc=True)
                    ready[pa] = fin
                    escbuf[pa] = eb

                def emit_av_pair(pa):
                    nonlocal last_av
                    eb = escbuf[pa]
                    cs = slice(256, 512) if pa == hp else slice(0, 512)
                    for hi in range(2):
                        kt = 2 * pa + hi
                        waits = ([ready[pa]] if hi == 0 else []) \
                            + [vt_ready[kt][0], vt_ready[kt][1]]
                        if kt == 0:
                            waits = waits + av_war[avj % 2]
                            av_war[avj % 2] = []
                        op = g.op("pe", mm(avbank[0:65, cs], vt[:, kt, h, :],
                                           esc_sb[eb][:, hi, cs],
                                           kt == 0, kt == n_kt - 1),
                                  waits, inc=True if kt == n_kt - 1 else None)
                        last_av = op
                    esc_war[eb] = [last_av]

                last_av = None
                for pa in range(min(2, n_pairs)):
                    emit_score_pair(pa)
                # threaded PE work sits between the scores and the AVs so the
                # exp+mask latency of the first pairs is hidden behind it, and
                # the rope/chain PE ops trail the DVE results they consume
                if pending_rope:
                    emit_b1_rope(pending_rope.pop(0))
                if pending_b2:
                    emit_b2_st(pending_b2.pop(0), next_slot_key())
                # defer half of qc3's units into window 3, which otherwise
                # has no threaded PE work to cover the scalar engine's exps
                has_unit = bool(pending_units) and not (qc == 2 and slot_idx >= 4)
                if has_unit:
                    if pending_d and slot_idx >= 1:
                        emit_d_group()
                    if n_pairs > 2:
                        emit_score_pair(2)
                    uqc, uwi, utt = pending_units.pop(0)
                    pending_rope.append(emit_b1_cop(
                        emit_b1_chain(uqc, uwi, utt, next_slot_key())))
                elif pending_d and slot_idx >= 1 and n_pairs > 2:
                    # no chain this slot: spread the out-projection group's
                    # matmuls around sc2 so neither sc2 nor sc3 outruns exp
                    emit_d_group(mid=lambda: emit_score_pair(2))
                else:
                    if n_pairs > 2:
                        emit_score_pair(2)
                    if pending_d and slot_idx >= 1:
                        emit_d_group()
                for pa in range(3, min(LOOKAHEAD, n_pairs)):
                    emit_score_pair(pa)
                nxtp = LOOKAHEAD
                for pa in range(n_pairs):
                    emit_av_pair(pa)
                    if nxtp < n_pairs:
                        emit_score_pair(nxtp)
                        nxtp += 1

                # normalization pipeline: reciprocal now; rb-broadcast DMA one
                # slot later; multiply (+ odd-half DMA) two slots later, so
                # the DMA latency never blocks the in-order DVE queue.
                myavj = avj
                rbsem = f"d_rb{myavj % 2}"
                rwaits = [last_av]
                if myavj >= 2:
                    rwaits.append((rbsem, 16 * (myavj // 2)))
                rop = g.op("dve",
                           lambda e, o=rcp_sb[myavj % 2], i=avbank:
                           e.reciprocal(o[64:65, :], i[64:65, :]),
                           rwaits, inc=True)

                def norm_rb(rop=rop, myavj=myavj, rbsem=rbsem):
                    rsrc = rcp_sb[myavj % 2][64:65, :]
                    bcast = bass.AP(tensor=rsrc.tensor, offset=rsrc.offset,
                                    ap=[rsrc.ap[0], [0, 64], rsrc.ap[1]])
                    dma("sp", rb_sb[myavj % 2][:, :], bcast, rbsem,
                        [rop, (rbsem, 16 * (myavj // 2))])

                def norm_mul(myavj=myavj, rbsem=rbsem, avbank=avbank,
                             p=p, half=half, qsl=qsl, myqc=qc):
                    nonlocal prev_mul, oddj
                    mwaits = [(rbsem, 16 * (myavj // 2 + 1))]
                    if prev_mul is not None:
                        mwaits.append(prev_mul)
                    if half == 0:
                        dst = attT[0:64, p, qsl]
                    else:
                        oddsem = f"d_odd{oddj % 2}"
                        if oddj >= 2:
                            mwaits.append((oddsem, 16 * (oddj // 2)))
                        dst = odd_sb[oddj % 2][:, :]
                    mop = g.op("dve",
                               lambda e, o=dst, a=avbank, r=rb_sb[myavj % 2]:
                               e.tensor_mul(o, a[0:64, :], r[:, :]),
                               mwaits, inc=True)
                    prev_mul = mop
                    qc_mul[myqc] = mop
                    if half == 1:
                        dma("sp", attT[64:128, p, qsl], odd_sb[oddj % 2][:, :],
                            oddsem, [mop, (oddsem, 16 * (oddj // 2))])
                        oddj += 1
                    av_war[myavj % 2] = [mop]

                pending_n1.append(norm_rb)
                pending_n2.append(norm_mul)
                avj += 1

                if qc == 3 and slot_idx >= 4 and pending_n1:
                    pending_n1.pop(0)()

            if qc + 2 <= 3:
                pending_units.extend(
                    (qc + 2, wi, tt) for tt in range(4) for wi in range(2))

            for st in range(4 * qc, 4 * qc + 4):
                for dc in range(2):
                    pending_d.append((st, dc, qc))

        while pending_n1:
            pending_n1.pop(0)()
        while pending_n2:
            pending_n2.pop(0)()
        while pending_rope:
            emit_b1_rope(pending_rope.pop(0))
        while pending_d:
            emit_d_group()

        g.resolve()

        with nc.allow_low_precision(reason="fp8/bf16 attention intermediates"), \
                nc.Block() as block:
            @block.tensor
            def _(eng):
                g.emit("pe", eng, sems)

            @block.scalar
            def _(eng):
                g.emit("act", eng, sems)

            @block.vector
            def _(eng):
                g.emit("dve", eng, sems)

            @block.gpsimd
            def _(eng):
                g.emit("gp", eng, sems)

            @block.sync
            def _(eng):
                g.emit("sp", eng, sems)

    return nc


def _get_nc():
    global _nc_cache
    if _nc_cache is None:
        _nc_cache = _build_nc()
    return _nc_cache


def _host_consts():
    perm = np.concatenate([
        h * HD + np.concatenate([np.arange(0, HD, 2), np.arange(1, HD, 2)])
        for h in range(8)
    ])
    P2 = np.zeros((128, 128), np.float32)
    for b0 in (0, 64):
        P2[np.arange(b0, b0 + 32), np.arange(b0 + 32, b0 + 64)] = 1.0
        P2[np.arange(b0 + 32, b0 + 64), np.arange(b0, b0 + 32)] = 1.0
    return perm, P2.astype(NPBF16)


def _host_tables(cos, sin):
    """cos/sin tables tiled to 128 partitions; the sin table carries the P2
    rotation's signs (-,+ per 32-row block) so the on-chip rotation is a
    pure partition-block swap."""
    cosr = np.ascontiguousarray(np.tile(cos.T, (4, 1))).astype(NPBF16)
    sgn = np.concatenate([-np.ones(32, np.float32), np.ones(32, np.float32)]
                         * 2)[:, None]
    sinr = np.ascontiguousarray(np.tile(sin.T, (4, 1)) * sgn).astype(NPBF16)
    return cosr, sinr


def _split8(a):
    """f32 -> (hi, lo) fp8e4m3 with lo = fp8(a - hi)."""
    hi = np.ascontiguousarray(a).astype(NPFP8)
    lo = (a - hi.astype(np.float32)).astype(NPFP8)
    return hi, lo


def kernel(x, freqs_cos, freqs_sin, wq, wk, wv, wo):
    global last_results
    x = np.asarray(x, np.float32)
    cos = np.asarray(freqs_cos, np.float32)
    sin = np.asarray(freqs_sin, np.float32)
    wq = np.asarray(wq, np.float32)
    wk = np.asarray(wk, np.float32)
    wv = np.asarray(wv, np.float32)
    wo = np.asarray(wo, np.float32)

    perm, protT = _host_consts()
    cosr, sinr = _host_tables(cos, sin)

    xs = [_split8(np.ascontiguousarray(x[b].T)) for b in range(B)]
    wqs, wks, wvs, wos = {}, {}, {}, {}
    for gg in range(2):
        gsl = slice(gg * HG, (gg + 1) * HG)
        wqs[gg] = _split8(np.ascontiguousarray((wq[gsl][perm] * WS).T))
        wks[gg] = _split8(np.ascontiguousarray((wk[gsl][perm] * WS).T))
        wvs[gg] = _split8(np.ascontiguousarray((wv[gsl] * WS).T))
        wos[gg] = np.ascontiguousarray(wo.T[gsl]).astype(NPBF16)

    in_maps = []
    for c in range(N_CORES):
        b, gg = c // 2, c % 2
        in_maps.append({
            "xh": xs[b][0], "xl": xs[b][1],
            "wqh": wqs[gg][0], "wql": wqs[gg][1],
            "wkh": wks[gg][0], "wkl": wks[gg][1],
            "wvh": wvs[gg][0], "wvl": wvs[gg][1],
            "woT": wos[gg],
            "cosr": cosr,
            "sinr": sinr,
            "protT": protT,
        })

    nc = _get_nc()
    last_results = run_bass_kernel_spmd(nc, in_maps, list(range(N_CORES)))
    res = last_results.results

    out = np.empty((B, S, D), np.float32)
    for b in range(B):
        out[b] = res[2 * b]["out"] + res[2 * b + 1]["out"]
    return out
